# revision 49
# baseline (speedup 1.0000x reference)
"""Causal self-attention with RoPE on 8 TRN2 NeuronCores.

Head-parallel tensor parallelism: core i owns heads 2i, 2i+1. Each core
computes its slice of the qkv projection (bf16 operands, fp32 psum),
per-head causal attention in SBUF with diagonal-block slicing, then the
normalized attention outputs are exchanged with a per-(batch, head)
AllToAll so every core performs the full output projection for its own
512-token shard.

Scheduling notes:
- all matmuls are bf16 (full PE rate; fp32r is no faster and costs 2x
  DMA/SBUF), PSUM always fp32.
- S blocks for one query chunk are emitted in groups >= 4 per shape to
  amortize the PE tile-geometry switch (~100 ns when alternating).
- exp() runs on the Scalar engine only (the only engine with exp); it is
  the attention-phase co-bottleneck, so Scalar does nothing else and S
  psum tiles pair two key-blocks so one activation covers [128, 1024].
- the qkv/rope/v-transpose work for batch 1 and the batch-0 projection
  are emitted as filler inside the (Scalar-bound) attention windows of
  batch 0 / batch 1 to keep the PE p-state ramped.
"""

import ml_dtypes
import numpy as np

import concourse.bass as bass
import concourse.mybir as mybir
import concourse.tile as tile
from concourse import bacc
from concourse.bass_utils import run_bass_kernel_spmd

F32 = mybir.dt.float32
BF16 = mybir.dt.bfloat16

B, T, C = 2, 2048, 1024
H, HD = 16, 64
NC = 8
HL = H // NC          # heads per core = 2
BT = B * T            # 4096
FQKV = 3 * HL * HD    # 384 rows of w_attn per core
TSH = BT // NC        # 512 output rows per core
NCH = BT // 512       # 8 column chunks of the [*, BT] activations
ROPE_BASE = 10000.0


DEBUG = False


def build():
    nc = bacc.Bacc(None, target_bir_lowering=False)

    xT_d = nc.dram_tensor("xT", [C, BT], BF16, kind="ExternalInput")
    wq_d = nc.dram_tensor("wqkvT", [C, FQKV], BF16, kind="ExternalInput")
    wp_d = nc.dram_tensor("wpT", [C, C], BF16, kind="ExternalInput")
    cos_d = nc.dram_tensor("cosT", [128, BT], BF16, kind="ExternalInput")
    sin_d = nc.dram_tensor("sinT", [128, BT], BF16, kind="ExternalInput")
    perm_d = nc.dram_tensor("permT", [128, 128], BF16, kind="ExternalInput")
    tri_d = nc.dram_tensor("tri", [128, 128], BF16, kind="ExternalInput")
    id_d = nc.dram_tensor("identF", [128, 128], F32, kind="ExternalInput")
    out_d = nc.dram_tensor("out", [TSH, C], F32, kind="ExternalOutput")
    dbg = {}
    if DEBUG:
        dbg['q0'] = nc.dram_tensor("dbg_q0", [128, 512], BF16,
                                   kind="ExternalOutput")
        dbg['k0'] = nc.dram_tensor("dbg_k0", [128, 512], BF16,
                                   kind="ExternalOutput")
        dbg['v0'] = nc.dram_tensor("dbg_v0", [128, 512], F32,
                                   kind="ExternalOutput")
        dbg['vsb0'] = nc.dram_tensor("dbg_vsb0", [128, 130], BF16,
                                     kind="ExternalOutput")
        dbg['pt0'] = nc.dram_tensor("dbg_pt0", [128, 1024], BF16,
                                    kind="ExternalOutput")
        dbg['av0'] = nc.dram_tensor("dbg_av0", [65, 512], F32,
                                    kind="ExternalOutput")
        dbg['st0'] = nc.dram_tensor("dbg_st0", [64, 512], BF16,
                                    kind="ExternalOutput")
        dbg['rc0'] = nc.dram_tensor("dbg_rc0", [1, 512], F32,
                                    kind="ExternalOutput")
        dbg['bcs0'] = nc.dram_tensor("dbg_bcs0", [64, 512], F32,
                                     kind="ExternalOutput")
        dbg['a2a0'] = nc.dram_tensor("dbg_a2a0", [512, 256], BF16,
                                     kind="ExternalOutput")

    # AllToAll staging. Batch 0: one merged exchange (block j rows
    # 128j:128j+128 = this core's 128 channels for core j's 256 tokens).
    # Batch 1 is split per head (block j rows 64j:64j+64) so the h0
    # exchange overlaps the h1 attention and only h1's is exposed.
    a2a_in = {0: nc.dram_tensor("a2ain0", [NC * 128, T // NC], BF16),
              (1, 0): nc.dram_tensor("a2ain10", [NC * HD, T // NC], BF16),
              (1, 1): nc.dram_tensor("a2ain11", [NC * HD, T // NC], BF16)}
    a2a_out = {0: nc.dram_tensor("a2aout0", [NC * 128, T // NC], BF16),
               (1, 0): nc.dram_tensor("a2aout10", [NC * HD, T // NC], BF16),
               (1, 1): nc.dram_tensor("a2aout11", [NC * HD, T // NC], BF16)}

    with tile.TileContext(nc) as tc:
        with (
            tc.tile_pool(name="persist", bufs=1) as pp,
            tc.tile_pool(name="work", bufs=2) as wk,
            tc.tile_pool(name="xtp", bufs=2) as xtp,
            tc.tile_pool(name="ptp", bufs=1) as ptp,
            tc.tile_pool(name="ps", bufs=1, space="PSUM") as ps,
        ):
            # ---------- constants / weights ----------
            wq_sb = []
            for c in range(8):
                t = pp.tile([128, FQKV], BF16, name=f"wq{c}", tag=f"wq{c}")
                nc.gpsimd.dma_start(t[:], wq_d[c * 128:(c + 1) * 128, :])
                wq_sb.append(t)
            perm_sb = pp.tile([128, 128], BF16, name="perm_sb", tag="perm_sb")
            nc.gpsimd.dma_start(perm_sb[:], perm_d[:])
            id_sb = pp.tile([128, 128], F32, name="id_sb", tag="id_sb")
            nc.gpsimd.dma_start(id_sb[:], id_d[:])
            tri_sb = pp.tile([128, 128], BF16, name="tri_sb", tag="tri_sb")
            nc.gpsimd.dma_start(tri_sb[:], tri_d[:])
            cos_sb = pp.tile([128, BT], BF16, name="cos_sb", tag="cos_sb")
            nc.gpsimd.dma_start(cos_sb[:], cos_d[:])
            sin_sb = pp.tile([128, BT], BF16, name="sin_sb", tag="sin_sb")
            nc.gpsimd.dma_start(sin_sb[:], sin_d[:])
            # w_proj tiles declared here; DMAs deferred until phase 1 has
            # its HBM bandwidth (wp is not needed until the projection)
            wp_sb = [pp.tile([128, C], BF16, name=f"wp{c}", tag=f"wp{c}")
                     for c in range(8)]

            def wp_load():
                for c in range(8):
                    nc.gpsimd.dma_start(wp_sb[c][:],
                                        wp_d[c * 128:(c + 1) * 128, :])

            onesf = pp.tile([128, 64], F32, name="onesf", tag="onesf")
            nc.vector.memset(onesf[:], 1.0)
            ones_c = pp.tile([128, 1], BF16, name="ones_c", tag="ones_c")
            nc.vector.tensor_copy(ones_c[:], onesf[:, 0:1])
            # all-ones [65,64]; row 64 is the partition-64-aligned stationary
            # for the denominator-broadcast matmul
            ones65 = pp.tile([65, 64], BF16, name="ones65", tag="ones65")
            nc.vector.tensor_copy(ones65[:], onesf[0:65, :])

            # ---------- PSUM slots ----------
            psS = [ps.tile([128, 1024], F32, name=f"psS{i}", tag=f"psS{i}")
                   for i in range(2)]
            av = [ps.tile([65, 512], F32, name=f"av{i}", tag=f"av{i}")
                  for i in range(2)]
            bcp = ps.tile([128, 512], F32, name="bcp", tag="bcp")
            pqp = ps.tile([128, 512], F32, name="pqp", tag="pqp")

            def ps_slot(i):
                """Six [128,512] qkv psum regions for one t-quarter."""
                if i < 4:
                    return psS[i // 2][:, (i % 2) * 512:(i % 2 + 1) * 512]
                return (bcp if i == 4 else pqp)[:]

            # ---------- activation chunks ----------
            qtc = [pp.tile([128, 512], BF16, name=f"qtc{i}", tag=f"qtc{i}")
                   for i in range(NCH)]
            ktc = [pp.tile([128, 512], BF16, name=f"ktc{i}", tag=f"ktc{i}")
                   for i in range(NCH)]
            vtc = [pp.tile([128, 512], F32, name=f"vtc{i}", tag=f"vtc{i}")
                   for i in range(NCH)]
            fdst = [qtc, ktc, vtc]
            v_sb = [pp.tile([128, 130], BF16, name=f"v{kb}", tag=f"v{kb}")
                    for kb in range(BT // 128)]

            # ---------- phase 1 pieces (also used as attention filler) ----
            def load_xt(th):
                xt = []
                for c in range(8):
                    t = xtp.tile([128, 1024], BF16, name=f"xt{th}{c}",
                                 tag=f"xt{c}")
                    nc.sync.dma_start(t[:], xT_d[c * 128:(c + 1) * 128,
                                                 th * 1024:(th + 1) * 1024])
                    xt.append(t)
                return xt

            def qkv_quantum(th, xt, f, tq, slot=None):
                # psum->sbuf copy: Scalar pre-attention (idle then), DVE
                # when running as filler inside the exp-bound windows
                eng = nc.scalar if slot is None else nc.vector
                if slot is None:
                    slot = ps_slot(f * 2 + tq)
                for c in range(8):
                    nc.tensor.matmul(
                        slot,
                        wq_sb[c][:, f * 128:(f + 1) * 128],
                        xt[c][:, tq * 512:(tq + 1) * 512],
                        start=(c == 0), stop=(c == 7),
                    )
                ch = th * 2 + tq
                if eng is nc.scalar:
                    eng.copy(fdst[f][ch][:], slot)
                else:
                    eng.tensor_copy(fdst[f][ch][:], slot)

            def rope_quantum(ch, slots=None):
                """RoPE in place on q and k chunk ch."""
                if slots is None:
                    slots = [pqp[:], pqp[:]]
                for which, tcl in ((0, qtc), (1, ktc)):
                    pr = slots[which]
                    nc.tensor.matmul(pr, perm_sb[:], tcl[ch][:],
                                     start=True, stop=True)
                    rot = wk.tile([128, 512], BF16, name=f"rot{which}{ch}",
                                  tag="rot")
                    nc.vector.tensor_mul(
                        rot[:], pr, sin_sb[:, ch * 512:(ch + 1) * 512])
                    nc.vector.tensor_mul(
                        tcl[ch][:], tcl[ch][:],
                        cos_sb[:, ch * 512:(ch + 1) * 512])
                    nc.vector.tensor_add(tcl[ch][:], tcl[ch][:], rot[:])

            def vt_quantum(ch, slots=None):
                """Transpose v chunk ch into 4 v_sb key blocks."""
                if slots is None:
                    slots = [bcp[:, 0:128], bcp[:, 128:256]]
                for j in range(4):
                    kb = ch * 4 + j
                    pv = slots[j % len(slots)]
                    nc.tensor.transpose(pv,
                                        vtc[ch][:, j * 128:(j + 1) * 128],
                                        id_sb[:])
                    v = v_sb[kb]
                    nc.vector.tensor_copy(v[:, 0:64], pv[:, 0:64])
                    nc.vector.tensor_copy(v[:, 65:129], pv[:, 64:128])
                    nc.vector.tensor_copy(v[:, 64:65], ones_c[:])
                    nc.vector.tensor_copy(v[:, 129:130], ones_c[:])

            # ---------- attention ----------
            pt_tiles = [ptp.tile([128, 1024], BF16, name=f"pt{i}",
                                 tag=f"pt{i}") for i in range(6)]
            pt_idx = [0]

            def attn_head(b, h, filler):
                hp = h * 64
                pending_norm = [None]

                def s_pair(pi, qc, kb_a, kb_b):
                    """S + exp for key blocks kb_a, kb_b of query chunk qc.
                    kb_b may be None. Returns (pt, [(kb, off, F), ...])."""
                    sp = psS[pi % 2]
                    pt = pt_tiles[pt_idx[0] % 6]
                    pt_idx[0] += 1
                    ent = []
                    for slot_i, kb in enumerate((kb_a, kb_b)):
                        if kb is None:
                            continue
                        koff = max(0, (kb - 4 * qc) * 128)
                        F = 512 - koff
                        off = slot_i * 512
                        kch = ktc[b * 4 + kb // 4]
                        nc.tensor.matmul(
                            sp[:, off:off + F],
                            kch[hp:hp + 64, (kb % 4) * 128:(kb % 4 + 1) * 128],
                            qtc[b * 4 + qc][hp:hp + 64, koff:512],
                            start=True, stop=True,
                        )
                        ent.append((kb, off, F, koff))
                    diag = ent[0][3] > 0 or (len(ent) > 1 and ent[1][3] > 0) \
                        or any(kb >= 4 * qc for kb, _, _, _ in ent)
                    if not diag:
                        nc.scalar.activation(
                            pt[:, 0:1024], sp[:, 0:1024],
                            mybir.ActivationFunctionType.Exp, scale=0.125)
                    else:
                        for kb, off, F, koff in ent:
                            nc.scalar.activation(
                                pt[:, off:off + F], sp[:, off:off + F],
                                mybir.ActivationFunctionType.Exp, scale=0.125)
                            if kb >= 4 * qc:
                                nc.vector.tensor_mul(
                                    pt[:, off:off + 128], pt[:, off:off + 128],
                                    tri_sb[:])
                    if DEBUG and b == 0 and h == 0 and qc == 0 and pi == 0:
                        nc.sync.dma_start(dbg['pt0'][:], pt[:])
                    return (pt, ent)

                def av_pair(avq, qc, pair, nkb):
                    pt, ent = pair
                    for kb, off, F, koff in ent:
                        nc.tensor.matmul(
                            avq[:, koff:512],
                            v_sb[b * 16 + kb][:, h * 65:(h + 1) * 65],
                            pt[:, off:off + F],
                            start=(kb == 0), stop=(kb == nkb - 1),
                        )

                for qc in range(4):
                    nkb = 4 * qc + 4
                    avq = av[qc % 2]
                    pairs = []
                    for p in range(0, nkb, 2):
                        pairs.append((p // 2, qc, p,
                                      p + 1 if p + 1 < nkb else None))
                    # emit S pairs and AV pairs in groups of 2 pairs (4 kb),
                    # AV lagging one group behind S
                    done = []
                    groups = [pairs[g:g + 2] for g in range(0, len(pairs), 2)]
                    prev = None
                    for gi, grp in enumerate(groups):
                        cur = [s_pair(*args) for args in grp]
                        if pending_norm[0] is not None:
                            pending_norm[0]()
                            pending_norm[0] = None
                        if prev is not None:
                            for pr_ in prev:
                                av_pair(avq, qc, pr_, nkb)
                        if filler:
                            filler(qc)
                        prev = cur
                    for pr_ in prev:
                        av_pair(avq, qc, pr_, nkb)

                    def norm(b=b, h=h, qc=qc, avq=avq):
                        if DEBUG and b == 0 and h == 0 and qc == 0:
                            avf = wk.tile([65, 512], F32, name="avf",
                                          tag="avf")
                            nc.vector.tensor_copy(avf[:], avq[:])
                            nc.sync.dma_start(dbg['av0'][:], avf[:])
                        # den (psum row 64) -> sbuf bf16 on its own
                        # partition, PE-broadcast to partitions 0:64, then
                        # reciprocal on the [64,512] base-0 region
                        dsb = wk.tile([65, 512], BF16, name=f"ds{b}{h}{qc}",
                                      tag="dsb")
                        # DVE, not Scalar: a scalar.copy here would thrash
                        # the ACT table set against exp (~2.7us per reload)
                        nc.vector.tensor_copy(dsb[64:65, :], avq[64:65, :])
                        nc.tensor.matmul(bcp[0:64, :], ones65[64:65, :],
                                         dsb[64:65, :],
                                         start=True, stop=True)
                        bcs = wk.tile([64, 512], F32, name=f"bcs{b}{h}{qc}",
                                      tag="bcs")
                        scr = wk.tile([64, 512], F32, name=f"scr{b}{h}{qc}",
                                      tag="scr")
                        nc.vector.reciprocal_approx_accurate(
                            out=bcs[:], in_=bcp[0:64, :], scratch=scr[:])
                        if DEBUG and b == 0 and h == 0 and qc == 0:
                            nc.sync.dma_start(dbg['rc0'][:], bcs[0:1, :])
                            nc.sync.dma_start(dbg['bcs0'][:], bcs[:])
                        st = wk.tile([64, 512], BF16, name=f"st{b}{h}{qc}",
                                     tag="st")
                        nc.vector.tensor_mul(st[:], avq[0:64, :], bcs[:])
                        if DEBUG and b == 0 and h == 0 and qc == 0:
                            nc.sync.dma_start(dbg['st0'][:], st[:])
                        for half in range(2):
                            if b == 0:
                                dst, r0 = a2a_in[0], 128 * (2 * qc + half) \
                                    + 64 * h
                            else:
                                dst, r0 = a2a_in[1, h], 64 * (2 * qc + half)
                            nc.sync.dma_start(
                                dst[r0:r0 + 64, :],
                                st[:, half * 256:(half + 1) * 256])
                    pending_norm[0] = norm
                pending_norm[0]()

            def a2a_issue(key):
                nc.gpsimd.collective_compute(
                    "AllToAll",
                    mybir.AluOpType.bypass,
                    replica_groups=[list(range(NC))],
                    ins=[a2a_in[key][:]],
                    outs=[a2a_out[key][:]],
                )

            # ---------- out-projection for this core's tokens ----------
            def proj_load(b):
                pl = []
                for c in range(8):
                    t = wk.tile([128, 256], BF16, name=f"pl{b}{c}",
                                tag=f"pl{c}")
                    if b == 0:
                        nc.gpsimd.dma_start(
                            t[:], a2a_out[0][c * 128:(c + 1) * 128, :])
                    else:
                        for h in range(2):
                            nc.gpsimd.dma_start(
                                t[h * 64:(h + 1) * 64, :],
                                a2a_out[1, h][c * 64:(c + 1) * 64, :])
                    pl.append(t)
                return pl

            def proj_quantum(b, pl, tb, oh):
                for c in range(8):
                    nc.tensor.matmul(
                        pqp[:],
                        pl[c][:, tb * 128:(tb + 1) * 128],
                        wp_sb[c][:, oh * 512:(oh + 1) * 512],
                        start=(c == 0), stop=(c == 7),
                    )
                pf = wk.tile([128, 512], F32, name=f"pf{b}{tb}{oh}",
                             tag="pf")
                nc.vector.tensor_copy(pf[:], pqp[:])
                nc.sync.dma_start(
                    out_d[b * 256 + tb * 128:b * 256 + (tb + 1) * 128,
                          oh * 512:(oh + 1) * 512], pf[:])

            # ================= schedule =================
            # phase 1 for batch 0 (th 0, 1)
            xt0 = load_xt(0)
            xt1 = load_xt(1)
            for f in range(3):
                for tq in range(2):
                    qkv_quantum(0, xt0, f, tq)
            for f in range(3):
                for tq in range(2):
                    qkv_quantum(1, xt1, f, tq)
            for ch in range(4):
                rope_quantum(ch, slots=[psS[0][:, 0:512], psS[1][:, 0:512]])
                vt_quantum(ch, slots=[psS[0][:, 512:640], psS[1][:, 512:640],
                                      bcp[:, 0:128], pqp[:, 0:128]])
            wp_load()
            if DEBUG:
                nc.sync.dma_start(dbg['q0'][:], qtc[0][:])
                nc.sync.dma_start(dbg['k0'][:], ktc[0][:])
                nc.sync.dma_start(dbg['v0'][:], vtc[0][:])
                nc.sync.dma_start(dbg['vsb0'][:], v_sb[0][:])

            # filler generator: batch-1 phase 1 then batch-0 projection
            fill_q = []
            xt2 = load_xt(2)
            for f in range(3):
                for tq in range(2):
                    fill_q.append(
                        lambda f=f, tq=tq: qkv_quantum(2, xt2, f, tq, pqp[:]))
            fill_q.append(lambda: rope_quantum(4))
            fill_q.append(lambda: vt_quantum(4))
            fill_q.append(lambda: rope_quantum(5))
            fill_q.append(lambda: vt_quantum(5))

            fill_q2 = []
            xt3 = [None]

            def load3():
                xt3[0] = load_xt(3)
            fill_q2.append(load3)
            for f in range(3):
                for tq in range(2):
                    fill_q2.append(
                        lambda f=f, tq=tq: qkv_quantum(3, xt3[0], f, tq,
                                                       pqp[:]))
            fill_q2.append(lambda: rope_quantum(6))
            fill_q2.append(lambda: vt_quantum(6))
            fill_q2.append(lambda: rope_quantum(7))
            fill_q2.append(lambda: vt_quantum(7))

            def mk_filler(queue, per_point):
                def filler(qc):
                    for _ in range(per_point):
                        if queue:
                            queue.pop(0)()
                return filler

            def drain(queue):
                while queue:
                    queue.pop(0)()

            # batch 0 attention; fill with batch-1 phase-1 work
            fill_q.extend(fill_q2)
            attn_head(0, 0, mk_filler(fill_q, 2))
            attn_head(0, 1, mk_filler(fill_q, 2))
            a2a_issue(0)
            drain(fill_q)

            # batch 1 attention; fill with batch-0 projection
            pl0 = [None]
            fill_p = []

            def loadp():
                pl0[0] = proj_load(0)
            fill_p.append(loadp)
            for tb in range(2):
                for oh in range(2):
                    fill_p.append(
                        lambda tb=tb, oh=oh: proj_quantum(0, pl0[0], tb, oh))

            attn_head(1, 0, None)
            a2a_issue((1, 0))
            attn_head(1, 1, mk_filler(fill_p, 1))
            a2a_issue((1, 1))
            drain(fill_p)

            pl1 = proj_load(1)
            for tb in range(2):
                for oh in range(2):
                    proj_quantum(1, pl1, tb, oh)
            if DEBUG:
                nc.sync.dma_start(dbg['a2a0'][:], a2a_out[0][:])

    nc.finalize()
    return nc


def host_inputs(x, w_attn, w_proj):
    """Host-side sharding/layout prep. Returns per-core in_maps."""
    bf = ml_dtypes.bfloat16
    x2 = np.ascontiguousarray(x.reshape(BT, C).T).astype(bf)  # [C,BT]

    inv = 1.0 / (ROPE_BASE ** (np.arange(0, HD, 2, dtype=np.float32) / HD))
    tpos = np.arange(T, dtype=np.float32)
    freqs = tpos[:, None] * inv[None, :]                  # [T, 32]
    emb = np.concatenate([freqs, freqs], axis=-1)         # [T, 64]
    cosT = np.cos(emb).T.astype(np.float32)               # [64, T]
    sinT = np.sin(emb).T.astype(np.float32)
    cos_full = np.ascontiguousarray(np.tile(cosT, (2, B))).astype(bf)
    sin_full = np.ascontiguousarray(np.tile(sinT, (2, B))).astype(bf)

    m64 = np.zeros((HD, HD), dtype=np.float32)
    half = HD // 2
    for d in range(half):
        m64[d, d + half] = -1.0
        m64[d + half, d] = 1.0
    perm = np.zeros((128, 128), dtype=np.float32)
    perm[0:HD, 0:HD] = m64
    perm[HD:128, HD:128] = m64
    permT = np.ascontiguousarray(perm.T).astype(bf)

    qi = np.arange(128)[None, :]
    ki = np.arange(128)[:, None]
    tri = np.ascontiguousarray((qi >= ki).astype(np.float32)).astype(bf)

    identF = np.eye(128, dtype=np.float32)
    wpT = np.ascontiguousarray(w_proj.T).astype(bf)

    in_maps = []
    for i in range(NC):
        r0 = i * (HL * HD)
        wq = w_attn[r0:r0 + HL * HD, :]
        wk_ = w_attn[C + r0:C + r0 + HL * HD, :]
        wv = w_attn[2 * C + r0:2 * C + r0 + HL * HD, :]
        wqkvT = np.ascontiguousarray(
            np.concatenate([wq, wk_, wv], axis=0).T).astype(bf)
        in_maps.append({
            "xT": x2, "wqkvT": wqkvT, "wpT": wpT,
            "cosT": cos_full, "sinT": sin_full, "permT": permT,
            "tri": tri, "identF": identF,
        })
    return in_maps


_NC_CACHE = None


def _get_nc():
    global _NC_CACHE
    if _NC_CACHE is None:
        _NC_CACHE = build()
    return _NC_CACHE


def run(x, w_attn, w_proj, trace=False):
    nc = _get_nc()
    in_maps = host_inputs(np.asarray(x), np.asarray(w_attn),
                          np.asarray(w_proj))
    res = run_bass_kernel_spmd(nc, in_maps, list(range(NC)), trace=trace)
    # core i returns [512, 1024]: rows 0:256 = batch0 rows [256i, 256i+256),
    # rows 256:512 = batch1 rows [256i, 256i+256)
    out = np.empty((B, T, C), dtype=np.float32)
    piece = T // NC
    for i in range(NC):
        sh = res.results[i]["out"]
        out[0, i * piece:(i + 1) * piece] = sh[0:piece]
        out[1, i * piece:(i + 1) * piece] = sh[piece:2 * piece]
    return out, res


def kernel(x, w_attn, w_proj):
    out, _ = run(x, w_attn, w_proj, trace=False)
    return out


# revision 56
# speedup vs baseline: 1.0288x; 1.0288x over previous
"""Causal self-attention with RoPE on 8 TRN2 NeuronCores.

Head-parallel tensor parallelism: core i owns heads 2i, 2i+1. Each core
computes its slice of the qkv projection (bf16 operands, fp32 psum),
per-head causal attention in SBUF with diagonal-block slicing, then the
normalized attention outputs are exchanged with a per-(batch, head)
AllToAll so every core performs the full output projection for its own
512-token shard.

Scheduling notes:
- all matmuls are bf16 (full PE rate; fp32r is no faster and costs 2x
  DMA/SBUF), PSUM always fp32.
- S blocks for one query chunk are emitted in groups >= 4 per shape to
  amortize the PE tile-geometry switch (~100 ns when alternating).
- exp() runs on the Scalar engine only (the only engine with exp); it is
  the attention-phase co-bottleneck, so Scalar does nothing else and S
  psum tiles pair two key-blocks so one activation covers [128, 1024].
- the qkv/rope/v-transpose work for batch 1 and the batch-0 projection
  are emitted as filler inside the (Scalar-bound) attention windows of
  batch 0 / batch 1 to keep the PE p-state ramped.
"""

import ml_dtypes
import numpy as np

import concourse.bass as bass
import concourse.mybir as mybir
import concourse.tile as tile
from concourse import bacc
from concourse.bass_utils import run_bass_kernel_spmd

F32 = mybir.dt.float32
BF16 = mybir.dt.bfloat16

B, T, C = 2, 2048, 1024
H, HD = 16, 64
NC = 8
HL = H // NC          # heads per core = 2
BT = B * T            # 4096
FQKV = 3 * HL * HD    # 384 rows of w_attn per core
TSH = BT // NC        # 512 output rows per core
NCH = BT // 512       # 8 column chunks of the [*, BT] activations
ROPE_BASE = 10000.0


DEBUG = False


def build():
    nc = bacc.Bacc(None, target_bir_lowering=False)

    xT_d = nc.dram_tensor("xT", [C, BT], BF16, kind="ExternalInput")
    wq_d = nc.dram_tensor("wqkvT", [C, FQKV], BF16, kind="ExternalInput")
    wp_d = nc.dram_tensor("wpT", [C, C], BF16, kind="ExternalInput")
    cos_d = nc.dram_tensor("cosT", [128, BT], BF16, kind="ExternalInput")
    sin_d = nc.dram_tensor("sinT", [128, BT], BF16, kind="ExternalInput")
    perm_d = nc.dram_tensor("permT", [128, 128], BF16, kind="ExternalInput")
    tri_d = nc.dram_tensor("tri", [128, 128], BF16, kind="ExternalInput")
    id_d = nc.dram_tensor("identF", [128, 128], F32, kind="ExternalInput")
    out_d = nc.dram_tensor("out", [TSH, C], F32, kind="ExternalOutput")
    dbg = {}
    if DEBUG:
        dbg['q0'] = nc.dram_tensor("dbg_q0", [128, 512], BF16,
                                   kind="ExternalOutput")
        dbg['k0'] = nc.dram_tensor("dbg_k0", [128, 512], BF16,
                                   kind="ExternalOutput")
        dbg['v0'] = nc.dram_tensor("dbg_v0", [128, 512], F32,
                                   kind="ExternalOutput")
        dbg['vsb0'] = nc.dram_tensor("dbg_vsb0", [128, 130], BF16,
                                     kind="ExternalOutput")
        dbg['pt0'] = nc.dram_tensor("dbg_pt0", [128, 1024], BF16,
                                    kind="ExternalOutput")
        dbg['av0'] = nc.dram_tensor("dbg_av0", [65, 512], F32,
                                    kind="ExternalOutput")
        dbg['st0'] = nc.dram_tensor("dbg_st0", [64, 512], BF16,
                                    kind="ExternalOutput")
        dbg['rc0'] = nc.dram_tensor("dbg_rc0", [1, 512], F32,
                                    kind="ExternalOutput")
        dbg['bcs0'] = nc.dram_tensor("dbg_bcs0", [64, 512], F32,
                                     kind="ExternalOutput")
        dbg['a2a0'] = nc.dram_tensor("dbg_a2a0", [512, 256], BF16,
                                     kind="ExternalOutput")

    # AllToAll staging. Batch 0: one merged exchange (block j rows
    # 128j:128j+128 = this core's 128 channels for core j's 256 tokens).
    # Batch 1 is split per head (block j rows 64j:64j+64) so the h0
    # exchange overlaps the h1 attention and only h1's is exposed.
    a2a_in = {0: nc.dram_tensor("a2ain0", [NC * 128, T // NC], BF16),
              (1, 0): nc.dram_tensor("a2ain10", [NC * HD, T // NC], BF16),
              (1, 1): nc.dram_tensor("a2ain11", [NC * HD, T // NC], BF16)}
    a2a_out = {0: nc.dram_tensor("a2aout0", [NC * 128, T // NC], BF16),
               (1, 0): nc.dram_tensor("a2aout10", [NC * HD, T // NC], BF16),
               (1, 1): nc.dram_tensor("a2aout11", [NC * HD, T // NC], BF16)}
    # tiny warm-up collective: its barrier absorbs the inter-core launch
    # skew while phase 1 computes, so the real exchanges don't pay it
    warm_in = nc.dram_tensor("warm_in", [NC, 8], F32)
    warm_out = nc.dram_tensor("warm_out", [NC, 8], F32)

    with tile.TileContext(nc) as tc:
        with (
            tc.tile_pool(name="persist", bufs=1) as pp,
            tc.tile_pool(name="work", bufs=2) as wk,
            tc.tile_pool(name="xtp", bufs=2) as xtp,
            tc.tile_pool(name="ptp", bufs=1) as ptp,
            tc.tile_pool(name="ps", bufs=1, space="PSUM") as ps,
        ):
            # warm-up collective first: absorbs inter-core launch skew
            nc.gpsimd.collective_compute(
                "AllToAll",
                mybir.AluOpType.bypass,
                replica_groups=[list(range(NC))],
                ins=[warm_in[:]],
                outs=[warm_out[:]],
            )

            # ---------- constants / weights ----------
            wq_sb = []
            for c in range(8):
                t = pp.tile([128, FQKV], BF16, name=f"wq{c}", tag=f"wq{c}")
                nc.gpsimd.dma_start(t[:], wq_d[c * 128:(c + 1) * 128, :])
                wq_sb.append(t)
            perm_sb = pp.tile([128, 128], BF16, name="perm_sb", tag="perm_sb")
            nc.gpsimd.dma_start(perm_sb[:], perm_d[:])
            id_sb = pp.tile([128, 128], F32, name="id_sb", tag="id_sb")
            nc.gpsimd.dma_start(id_sb[:], id_d[:])
            tri_sb = pp.tile([128, 128], BF16, name="tri_sb", tag="tri_sb")
            nc.gpsimd.dma_start(tri_sb[:], tri_d[:])
            cos_sb = pp.tile([128, BT], BF16, name="cos_sb", tag="cos_sb")
            nc.gpsimd.dma_start(cos_sb[:], cos_d[:])
            sin_sb = pp.tile([128, BT], BF16, name="sin_sb", tag="sin_sb")
            nc.gpsimd.dma_start(sin_sb[:], sin_d[:])
            # w_proj tiles declared here; DMAs deferred until phase 1 has
            # its HBM bandwidth (wp is not needed until the projection)
            wp_sb = [pp.tile([128, C], BF16, name=f"wp{c}", tag=f"wp{c}")
                     for c in range(8)]

            def wp_load():
                for c in range(8):
                    nc.gpsimd.dma_start(wp_sb[c][:],
                                        wp_d[c * 128:(c + 1) * 128, :])

            onesf = pp.tile([128, 64], F32, name="onesf", tag="onesf")
            nc.vector.memset(onesf[:], 1.0)
            ones_c = pp.tile([128, 1], BF16, name="ones_c", tag="ones_c")
            nc.vector.tensor_copy(ones_c[:], onesf[:, 0:1])
            # all-ones [65,64]; row 64 is the partition-64-aligned stationary
            # for the denominator-broadcast matmul
            ones65 = pp.tile([65, 64], BF16, name="ones65", tag="ones65")
            nc.vector.tensor_copy(ones65[:], onesf[0:65, :])

            # ---------- PSUM slots ----------
            psS = [ps.tile([128, 1024], F32, name=f"psS{i}", tag=f"psS{i}")
                   for i in range(2)]
            av = [ps.tile([65, 512], F32, name=f"av{i}", tag=f"av{i}")
                  for i in range(2)]
            bcp = ps.tile([128, 512], F32, name="bcp", tag="bcp")
            pqp = ps.tile([128, 512], F32, name="pqp", tag="pqp")

            def ps_slot(i):
                """Six [128,512] qkv psum regions for one t-quarter."""
                if i < 4:
                    return psS[i // 2][:, (i % 2) * 512:(i % 2 + 1) * 512]
                return (bcp if i == 4 else pqp)[:]

            # ---------- activation chunks ----------
            qtc = [pp.tile([128, 512], BF16, name=f"qtc{i}", tag=f"qtc{i}")
                   for i in range(NCH)]
            ktc = [pp.tile([128, 512], BF16, name=f"ktc{i}", tag=f"ktc{i}")
                   for i in range(NCH)]
            vtc = [pp.tile([128, 512], F32, name=f"vtc{i}", tag=f"vtc{i}")
                   for i in range(NCH)]
            fdst = [qtc, ktc, vtc]
            v_sb = [pp.tile([128, 130], BF16, name=f"v{kb}", tag=f"v{kb}")
                    for kb in range(BT // 128)]

            # ---------- phase 1 pieces (also used as attention filler) ----
            def load_xt(th):
                xt = []
                for c in range(8):
                    t = xtp.tile([128, 1024], BF16, name=f"xt{th}{c}",
                                 tag=f"xt{c}")
                    nc.sync.dma_start(t[:], xT_d[c * 128:(c + 1) * 128,
                                                 th * 1024:(th + 1) * 1024])
                    xt.append(t)
                return xt

            def qkv_quantum(th, xt, f, tq, slot=None):
                # psum->sbuf copy: Scalar pre-attention (idle then), DVE
                # when running as filler inside the exp-bound windows
                eng = nc.scalar if slot is None else nc.vector
                if slot is None:
                    slot = ps_slot(f * 2 + tq)
                for c in range(8):
                    nc.tensor.matmul(
                        slot,
                        wq_sb[c][:, f * 128:(f + 1) * 128],
                        xt[c][:, tq * 512:(tq + 1) * 512],
                        start=(c == 0), stop=(c == 7),
                    )
                ch = th * 2 + tq
                if eng is nc.scalar:
                    eng.copy(fdst[f][ch][:], slot)
                else:
                    eng.tensor_copy(fdst[f][ch][:], slot)

            def rope_quantum(ch, slots=None):
                """RoPE in place on q and k chunk ch. Because the rope
                tables repeat across the rotate halves (sin[perm(d)] ==
                sin[d]), rot_half(x)*sin == perm @ (x*sin), so the sin
                multiply happens on SBUF bf16 before the matmul."""
                if slots is None:
                    slots = [pqp[:], pqp[:]]
                for which, tcl in ((0, qtc), (1, ktc)):
                    pr = slots[which]
                    xs = wk.tile([128, 512], BF16, name=f"xs{which}{ch}",
                                 tag="xs")
                    nc.vector.tensor_mul(
                        xs[:], tcl[ch][:], sin_sb[:, ch * 512:(ch + 1) * 512])
                    nc.tensor.matmul(pr, perm_sb[:], xs[:],
                                     start=True, stop=True)
                    nc.vector.tensor_mul(
                        tcl[ch][:], tcl[ch][:],
                        cos_sb[:, ch * 512:(ch + 1) * 512])
                    nc.vector.tensor_add(tcl[ch][:], tcl[ch][:], pr)

            def vt_quantum(ch, slots=None, eng=None):
                """Transpose v chunk ch into 4 v_sb key blocks. eng: engine
                for psum->sbuf copies (scalar pre-attention only — its
                copies are safe before the first exp's table load)."""
                if slots is None:
                    slots = [bcp[:, 0:128], bcp[:, 128:256]]
                if eng is None:
                    eng = nc.vector
                for j in range(4):
                    kb = ch * 4 + j
                    pv = slots[j % len(slots)]
                    nc.tensor.transpose(pv,
                                        vtc[ch][:, j * 128:(j + 1) * 128],
                                        id_sb[:])
                    v = v_sb[kb]
                    if eng is nc.scalar:
                        eng.copy(v[:, 0:64], pv[:, 0:64])
                        eng.copy(v[:, 65:129], pv[:, 64:128])
                    else:
                        eng.tensor_copy(v[:, 0:64], pv[:, 0:64])
                        eng.tensor_copy(v[:, 65:129], pv[:, 64:128])
                    nc.gpsimd.tensor_copy(v[:, 64:65], ones_c[:])
                    nc.gpsimd.tensor_copy(v[:, 129:130], ones_c[:])

            # ---------- attention ----------
            pt_tiles = [ptp.tile([128, 1024], BF16, name=f"pt{i}",
                                 tag=f"pt{i}") for i in range(6)]
            pt_idx = [0]

            def attn_head(b, h, filler):
                hp = h * 64
                pending_norm = [None]

                def s_pair(pi, qc, kb_a, kb_b):
                    """S + exp for key blocks kb_a, kb_b of query chunk qc.
                    kb_b may be None. Returns (pt, [(kb, off, F), ...])."""
                    sp = psS[pi % 2]
                    pt = pt_tiles[pt_idx[0] % 6]
                    pt_idx[0] += 1
                    ent = []
                    for slot_i, kb in enumerate((kb_a, kb_b)):
                        if kb is None:
                            continue
                        koff = max(0, (kb - 4 * qc) * 128)
                        F = 512 - koff
                        off = slot_i * 512
                        kch = ktc[b * 4 + kb // 4]
                        nc.tensor.matmul(
                            sp[:, off:off + F],
                            kch[hp:hp + 64, (kb % 4) * 128:(kb % 4 + 1) * 128],
                            qtc[b * 4 + qc][hp:hp + 64, koff:512],
                            start=True, stop=True,
                        )
                        ent.append((kb, off, F, koff))
                    diag = ent[0][3] > 0 or (len(ent) > 1 and ent[1][3] > 0) \
                        or any(kb >= 4 * qc for kb, _, _, _ in ent)
                    if not diag:
                        nc.scalar.activation(
                            pt[:, 0:1024], sp[:, 0:1024],
                            mybir.ActivationFunctionType.Exp, scale=0.125)
                    else:
                        for kb, off, F, koff in ent:
                            nc.scalar.activation(
                                pt[:, off:off + F], sp[:, off:off + F],
                                mybir.ActivationFunctionType.Exp, scale=0.125)
                            if kb >= 4 * qc:
                                nc.vector.tensor_mul(
                                    pt[:, off:off + 128], pt[:, off:off + 128],
                                    tri_sb[:])
                    if DEBUG and b == 0 and h == 0 and qc == 0 and pi == 0:
                        nc.sync.dma_start(dbg['pt0'][:], pt[:])
                    return (pt, ent)

                def av_pair(avq, qc, pair, nkb):
                    pt, ent = pair
                    for kb, off, F, koff in ent:
                        nc.tensor.matmul(
                            avq[:, koff:512],
                            v_sb[b * 16 + kb][:, h * 65:(h + 1) * 65],
                            pt[:, off:off + F],
                            start=(kb == 0), stop=(kb == nkb - 1),
                        )

                for qc in range(4):
                    nkb = 4 * qc + 4
                    avq = av[qc % 2]
                    pairs = []
                    for p in range(0, nkb, 2):
                        pairs.append((p // 2, qc, p,
                                      p + 1 if p + 1 < nkb else None))
                    # emit S pairs and AV pairs in groups of 2 pairs (4 kb),
                    # AV lagging one group behind S
                    done = []
                    groups = [pairs[g:g + 2] for g in range(0, len(pairs), 2)]
                    prev = None
                    for gi, grp in enumerate(groups):
                        cur = [s_pair(*args) for args in grp]
                        if pending_norm[0] is not None:
                            pending_norm[0]()
                            pending_norm[0] = None
                        if prev is not None:
                            for pr_ in prev:
                                av_pair(avq, qc, pr_, nkb)
                        if filler:
                            filler(qc)
                        prev = cur
                    for pr_ in prev:
                        av_pair(avq, qc, pr_, nkb)

                    def norm(b=b, h=h, qc=qc, avq=avq):
                        if DEBUG and b == 0 and h == 0 and qc == 0:
                            avf = wk.tile([65, 512], F32, name="avf",
                                          tag="avf")
                            nc.vector.tensor_copy(avf[:], avq[:])
                            nc.sync.dma_start(dbg['av0'][:], avf[:])
                        # den (psum row 64) -> sbuf bf16 on its own
                        # partition, PE-broadcast to partitions 0:64, then
                        # reciprocal on the [64,512] base-0 region
                        dsb = wk.tile([65, 512], BF16, name=f"ds{b}{h}{qc}",
                                      tag="dsb")
                        # DVE, not Scalar: a scalar.copy here would thrash
                        # the ACT table set against exp (~2.7us per reload)
                        nc.vector.tensor_copy(dsb[64:65, :], avq[64:65, :])
                        nc.tensor.matmul(bcp[0:64, :], ones65[64:65, :],
                                         dsb[64:65, :],
                                         start=True, stop=True)
                        bcs = wk.tile([64, 512], F32, name=f"bcs{b}{h}{qc}",
                                      tag="bcs")
                        scr = wk.tile([64, 512], F32, name=f"scr{b}{h}{qc}",
                                      tag="scr")
                        nc.vector.reciprocal_approx_accurate(
                            out=bcs[:], in_=bcp[0:64, :], scratch=scr[:])
                        if DEBUG and b == 0 and h == 0 and qc == 0:
                            nc.sync.dma_start(dbg['rc0'][:], bcs[0:1, :])
                            nc.sync.dma_start(dbg['bcs0'][:], bcs[:])
                        st = wk.tile([64, 512], BF16, name=f"st{b}{h}{qc}",
                                     tag="st")
                        nc.vector.tensor_mul(st[:], avq[0:64, :], bcs[:])
                        if DEBUG and b == 0 and h == 0 and qc == 0:
                            nc.sync.dma_start(dbg['st0'][:], st[:])
                        for half in range(2):
                            if b == 0:
                                dst, r0 = a2a_in[0], 128 * (2 * qc + half) \
                                    + 64 * h
                            else:
                                dst, r0 = a2a_in[1, h], 64 * (2 * qc + half)
                            nc.sync.dma_start(
                                dst[r0:r0 + 64, :],
                                st[:, half * 256:(half + 1) * 256])
                    pending_norm[0] = norm
                pending_norm[0]()

            def a2a_issue(key):
                nc.gpsimd.collective_compute(
                    "AllToAll",
                    mybir.AluOpType.bypass,
                    replica_groups=[list(range(NC))],
                    ins=[a2a_in[key][:]],
                    outs=[a2a_out[key][:]],
                )

            # ---------- out-projection for this core's tokens ----------
            def proj_load(b):
                pl = []
                for c in range(8):
                    t = wk.tile([128, 256], BF16, name=f"pl{b}{c}",
                                tag=f"pl{c}")
                    if b == 0:
                        nc.gpsimd.dma_start(
                            t[:], a2a_out[0][c * 128:(c + 1) * 128, :])
                    else:
                        for h in range(2):
                            nc.gpsimd.dma_start(
                                t[h * 64:(h + 1) * 64, :],
                                a2a_out[1, h][c * 64:(c + 1) * 64, :])
                    pl.append(t)
                return pl

            def proj_quantum(b, pl, tb, oh, slot=None):
                po = slot if slot is not None else pqp[:]
                for c in range(8):
                    nc.tensor.matmul(
                        po,
                        pl[c][:, tb * 128:(tb + 1) * 128],
                        wp_sb[c][:, oh * 512:(oh + 1) * 512],
                        start=(c == 0), stop=(c == 7),
                    )
                pf = wk.tile([128, 512], F32, name=f"pf{b}{tb}{oh}",
                             tag="pf")
                nc.vector.tensor_copy(pf[:], po)
                nc.sync.dma_start(
                    out_d[b * 256 + tb * 128:b * 256 + (tb + 1) * 128,
                          oh * 512:(oh + 1) * 512], pf[:])

            # ================= schedule =================
            # phase 1 for batch 0 (th 0, 1)
            xt0 = load_xt(0)
            xt1 = load_xt(1)
            for f in range(3):
                for tq in range(2):
                    qkv_quantum(0, xt0, f, tq)
            for f in range(3):
                for tq in range(2):
                    qkv_quantum(1, xt1, f, tq)
            for ch in range(4):
                rope_quantum(ch, slots=[psS[0][:, 0:512], psS[1][:, 0:512]])
                vt_quantum(ch, slots=[psS[0][:, 512:640], psS[1][:, 512:640],
                                      bcp[:, 0:128], pqp[:, 0:128]],
                           eng=nc.scalar)
            wp_load()
            if DEBUG:
                nc.sync.dma_start(dbg['q0'][:], qtc[0][:])
                nc.sync.dma_start(dbg['k0'][:], ktc[0][:])
                nc.sync.dma_start(dbg['v0'][:], vtc[0][:])
                nc.sync.dma_start(dbg['vsb0'][:], v_sb[0][:])

            # filler generator: batch-1 phase 1 then batch-0 projection
            fill_q = []
            xt2 = load_xt(2)
            for f in range(3):
                for tq in range(2):
                    fill_q.append(
                        lambda f=f, tq=tq: qkv_quantum(2, xt2, f, tq, pqp[:]))
            fill_q.append(lambda: rope_quantum(4))
            fill_q.append(lambda: vt_quantum(4))
            fill_q.append(lambda: rope_quantum(5))
            fill_q.append(lambda: vt_quantum(5))

            fill_q2 = []
            xt3 = [None]

            def load3():
                xt3[0] = load_xt(3)
            fill_q2.append(load3)
            for f in range(3):
                for tq in range(2):
                    fill_q2.append(
                        lambda f=f, tq=tq: qkv_quantum(3, xt3[0], f, tq,
                                                       pqp[:]))
            fill_q2.append(lambda: rope_quantum(6))
            fill_q2.append(lambda: vt_quantum(6))
            fill_q2.append(lambda: rope_quantum(7))
            fill_q2.append(lambda: vt_quantum(7))

            def mk_filler(queue, per_point):
                def filler(qc):
                    for _ in range(per_point):
                        if queue:
                            queue.pop(0)()
                return filler

            def drain(queue):
                while queue:
                    queue.pop(0)()

            # batch 0 attention; fill with batch-1 phase-1 work
            fill_q.extend(fill_q2)
            attn_head(0, 0, mk_filler(fill_q, 2))
            attn_head(0, 1, mk_filler(fill_q, 2))
            a2a_issue(0)
            drain(fill_q)

            # batch 1 attention; fill with batch-0 projection and the
            # early (h0) half of batch-1's projection inputs
            pl0 = [None]
            fill_p = []

            def loadp():
                pl0[0] = proj_load(0)
            fill_p.append(loadp)
            for tb in range(2):
                for oh in range(2):
                    fill_p.append(
                        lambda tb=tb, oh=oh: proj_quantum(0, pl0[0], tb, oh))

            pl1 = [wk.tile([128, 256], BF16, name=f"pl1{c}", tag=f"pl{c}")
                   for c in range(8)]

            def load1_h0():
                for c in range(8):
                    nc.gpsimd.dma_start(
                        pl1[c][0:64, :], a2a_out[1, 0][c * 64:(c + 1) * 64, :])

            attn_head(1, 0, None)
            a2a_issue((1, 0))
            fill_p.append(load1_h0)
            attn_head(1, 1, mk_filler(fill_p, 1))
            a2a_issue((1, 1))
            drain(fill_p)

            # tail: h1 rows of proj inputs across three queues, proj on
            # the psS slots (free after attention) to avoid serialization
            engs = [nc.gpsimd, nc.sync, nc.scalar]
            for c in range(8):
                engs[c % 3].dma_start(
                    pl1[c][64:128, :], a2a_out[1, 1][c * 64:(c + 1) * 64, :])
            tail_slots = [psS[0][:, 0:512], psS[0][:, 512:1024],
                          psS[1][:, 0:512], psS[1][:, 512:1024]]
            for tb in range(2):
                for oh in range(2):
                    proj_quantum(1, pl1, tb, oh,
                                 slot=tail_slots[tb * 2 + oh])
            if DEBUG:
                nc.sync.dma_start(dbg['a2a0'][:], a2a_out[0][:])

    nc.finalize()
    return nc


def host_inputs(x, w_attn, w_proj):
    """Host-side sharding/layout prep. Returns per-core in_maps."""
    bf = ml_dtypes.bfloat16
    x2 = np.ascontiguousarray(x.reshape(BT, C).T).astype(bf)  # [C,BT]

    inv = 1.0 / (ROPE_BASE ** (np.arange(0, HD, 2, dtype=np.float32) / HD))
    tpos = np.arange(T, dtype=np.float32)
    freqs = tpos[:, None] * inv[None, :]                  # [T, 32]
    emb = np.concatenate([freqs, freqs], axis=-1)         # [T, 64]
    cosT = np.cos(emb).T.astype(np.float32)               # [64, T]
    sinT = np.sin(emb).T.astype(np.float32)
    cos_full = np.ascontiguousarray(np.tile(cosT, (2, B))).astype(bf)
    sin_full = np.ascontiguousarray(np.tile(sinT, (2, B))).astype(bf)

    m64 = np.zeros((HD, HD), dtype=np.float32)
    half = HD // 2
    for d in range(half):
        m64[d, d + half] = -1.0
        m64[d + half, d] = 1.0
    perm = np.zeros((128, 128), dtype=np.float32)
    perm[0:HD, 0:HD] = m64
    perm[HD:128, HD:128] = m64
    permT = np.ascontiguousarray(perm.T).astype(bf)

    qi = np.arange(128)[None, :]
    ki = np.arange(128)[:, None]
    tri = np.ascontiguousarray((qi >= ki).astype(np.float32)).astype(bf)

    identF = np.eye(128, dtype=np.float32)
    wpT = np.ascontiguousarray(w_proj.T).astype(bf)

    in_maps = []
    for i in range(NC):
        r0 = i * (HL * HD)
        wq = w_attn[r0:r0 + HL * HD, :]
        wk_ = w_attn[C + r0:C + r0 + HL * HD, :]
        wv = w_attn[2 * C + r0:2 * C + r0 + HL * HD, :]
        wqkvT = np.ascontiguousarray(
            np.concatenate([wq, wk_, wv], axis=0).T).astype(bf)
        in_maps.append({
            "xT": x2, "wqkvT": wqkvT, "wpT": wpT,
            "cosT": cos_full, "sinT": sin_full, "permT": permT,
            "tri": tri, "identF": identF,
        })
    return in_maps


_NC_CACHE = None


def _get_nc():
    global _NC_CACHE
    if _NC_CACHE is None:
        _NC_CACHE = build()
    return _NC_CACHE


def run(x, w_attn, w_proj, trace=False):
    nc = _get_nc()
    in_maps = host_inputs(np.asarray(x), np.asarray(w_attn),
                          np.asarray(w_proj))
    res = run_bass_kernel_spmd(nc, in_maps, list(range(NC)), trace=trace)
    # core i returns [512, 1024]: rows 0:256 = batch0 rows [256i, 256i+256),
    # rows 256:512 = batch1 rows [256i, 256i+256)
    out = np.empty((B, T, C), dtype=np.float32)
    piece = T // NC
    for i in range(NC):
        sh = res.results[i]["out"]
        out[0, i * piece:(i + 1) * piece] = sh[0:piece]
        out[1, i * piece:(i + 1) * piece] = sh[piece:2 * piece]
    return out, res


def kernel(x, w_attn, w_proj):
    out, _ = run(x, w_attn, w_proj, trace=False)
    return out


# revision 60
# speedup vs baseline: 1.0383x; 1.0092x over previous
"""Causal self-attention with RoPE on 8 TRN2 NeuronCores.

Head-parallel tensor parallelism: core i owns heads 2i, 2i+1. Each core
computes its slice of the qkv projection (bf16 operands, fp32 psum),
per-head causal attention in SBUF with diagonal-block slicing, then the
normalized attention outputs are exchanged with a per-(batch, head)
AllToAll so every core performs the full output projection for its own
512-token shard.

Scheduling notes:
- all matmuls are bf16 (full PE rate; fp32r is no faster and costs 2x
  DMA/SBUF), PSUM always fp32.
- S blocks for one query chunk are emitted in groups >= 4 per shape to
  amortize the PE tile-geometry switch (~100 ns when alternating).
- exp() runs on the Scalar engine only (the only engine with exp); it is
  the attention-phase co-bottleneck, so Scalar does nothing else and S
  psum tiles pair two key-blocks so one activation covers [128, 1024].
- the qkv/rope/v-transpose work for batch 1 and the batch-0 projection
  are emitted as filler inside the (Scalar-bound) attention windows of
  batch 0 / batch 1 to keep the PE p-state ramped.
"""

import ml_dtypes
import numpy as np

import concourse.bass as bass
import concourse.mybir as mybir
import concourse.tile as tile
from concourse import bacc
from concourse.bass_utils import run_bass_kernel_spmd

F32 = mybir.dt.float32
BF16 = mybir.dt.bfloat16

B, T, C = 2, 2048, 1024
H, HD = 16, 64
NC = 8
HL = H // NC          # heads per core = 2
BT = B * T            # 4096
FQKV = 3 * HL * HD    # 384 rows of w_attn per core
TSH = BT // NC        # 512 output rows per core
NCH = BT // 512       # 8 column chunks of the [*, BT] activations
ROPE_BASE = 10000.0


DEBUG = False


def build():
    nc = bacc.Bacc(None, target_bir_lowering=False)

    xT_d = nc.dram_tensor("xT", [C, BT], BF16, kind="ExternalInput")
    wq_d = nc.dram_tensor("wqkvT", [C, FQKV], BF16, kind="ExternalInput")
    wp_d = nc.dram_tensor("wpT", [C, C], BF16, kind="ExternalInput")
    cos_d = nc.dram_tensor("cosT", [128, BT], BF16, kind="ExternalInput")
    sin_d = nc.dram_tensor("sinT", [128, BT], BF16, kind="ExternalInput")
    perm_d = nc.dram_tensor("permT", [128, 128], BF16, kind="ExternalInput")
    tri_d = nc.dram_tensor("tri", [128, 128], BF16, kind="ExternalInput")
    id_d = nc.dram_tensor("identF", [128, 128], F32, kind="ExternalInput")
    out_d = nc.dram_tensor("out", [TSH, C], F32, kind="ExternalOutput")
    dbg = {}
    if DEBUG:
        dbg['q0'] = nc.dram_tensor("dbg_q0", [128, 512], BF16,
                                   kind="ExternalOutput")
        dbg['k0'] = nc.dram_tensor("dbg_k0", [128, 512], BF16,
                                   kind="ExternalOutput")
        dbg['v0'] = nc.dram_tensor("dbg_v0", [128, 512], F32,
                                   kind="ExternalOutput")
        dbg['vsb0'] = nc.dram_tensor("dbg_vsb0", [128, 130], BF16,
                                     kind="ExternalOutput")
        dbg['pt0'] = nc.dram_tensor("dbg_pt0", [128, 1024], BF16,
                                    kind="ExternalOutput")
        dbg['av0'] = nc.dram_tensor("dbg_av0", [65, 512], F32,
                                    kind="ExternalOutput")
        dbg['st0'] = nc.dram_tensor("dbg_st0", [64, 512], BF16,
                                    kind="ExternalOutput")
        dbg['rc0'] = nc.dram_tensor("dbg_rc0", [1, 512], F32,
                                    kind="ExternalOutput")
        dbg['bcs0'] = nc.dram_tensor("dbg_bcs0", [64, 512], F32,
                                     kind="ExternalOutput")
        dbg['a2a0'] = nc.dram_tensor("dbg_a2a0", [512, 256], BF16,
                                     kind="ExternalOutput")

    den_dram = {(b_, h_, qc_): nc.dram_tensor(f"dend{b_}{h_}{qc_}",
                                              [1, 512], F32)
                for b_ in range(2) for h_ in range(2) for qc_ in range(4)}
    # AllToAll staging. Batch 0: one merged exchange (block j rows
    # 128j:128j+128 = this core's 128 channels for core j's 256 tokens).
    # Batch 1 is split per head (block j rows 64j:64j+64) so the h0
    # exchange overlaps the h1 attention and only h1's is exposed.
    a2a_in = {0: nc.dram_tensor("a2ain0", [NC * 128, T // NC], BF16),
              (1, 0): nc.dram_tensor("a2ain10", [NC * HD, T // NC], BF16),
              (1, 1): nc.dram_tensor("a2ain11", [NC * HD, T // NC], BF16)}
    a2a_out = {0: nc.dram_tensor("a2aout0", [NC * 128, T // NC], BF16),
               (1, 0): nc.dram_tensor("a2aout10", [NC * HD, T // NC], BF16),
               (1, 1): nc.dram_tensor("a2aout11", [NC * HD, T // NC], BF16)}
    # tiny warm-up collective: its barrier absorbs the inter-core launch
    # skew while phase 1 computes, so the real exchanges don't pay it
    warm_in = nc.dram_tensor("warm_in", [NC, 8], F32)
    warm_out = nc.dram_tensor("warm_out", [NC, 8], F32)

    with tile.TileContext(nc) as tc:
        with (
            tc.tile_pool(name="persist", bufs=1) as pp,
            tc.tile_pool(name="work", bufs=2) as wk,
            tc.tile_pool(name="xtp", bufs=2) as xtp,
            tc.tile_pool(name="ptp", bufs=1) as ptp,
            tc.tile_pool(name="ps", bufs=1, space="PSUM") as ps,
        ):
            # warm-up collective first: absorbs inter-core launch skew
            nc.gpsimd.collective_compute(
                "AllToAll",
                mybir.AluOpType.bypass,
                replica_groups=[list(range(NC))],
                ins=[warm_in[:]],
                outs=[warm_out[:]],
            )

            # ---------- constants / weights ----------
            wq_sb = []
            for c in range(8):
                t = pp.tile([128, FQKV], BF16, name=f"wq{c}", tag=f"wq{c}")
                nc.gpsimd.dma_start(t[:], wq_d[c * 128:(c + 1) * 128, :])
                wq_sb.append(t)
            perm_sb = pp.tile([128, 128], BF16, name="perm_sb", tag="perm_sb")
            nc.gpsimd.dma_start(perm_sb[:], perm_d[:])
            id_sb = pp.tile([128, 128], F32, name="id_sb", tag="id_sb")
            nc.gpsimd.dma_start(id_sb[:], id_d[:])
            tri_sb = pp.tile([128, 128], BF16, name="tri_sb", tag="tri_sb")
            nc.gpsimd.dma_start(tri_sb[:], tri_d[:])
            cos_sb = pp.tile([128, BT], BF16, name="cos_sb", tag="cos_sb")
            nc.gpsimd.dma_start(cos_sb[:], cos_d[:])
            sin_sb = pp.tile([128, BT], BF16, name="sin_sb", tag="sin_sb")
            nc.gpsimd.dma_start(sin_sb[:], sin_d[:])
            # w_proj tiles declared here; DMAs deferred until phase 1 has
            # its HBM bandwidth (wp is not needed until the projection)
            wp_sb = [pp.tile([128, C], BF16, name=f"wp{c}", tag=f"wp{c}")
                     for c in range(8)]

            def wp_load():
                for c in range(8):
                    nc.gpsimd.dma_start(wp_sb[c][:],
                                        wp_d[c * 128:(c + 1) * 128, :])

            onesf = pp.tile([128, 64], F32, name="onesf", tag="onesf")
            nc.vector.memset(onesf[:], 1.0)
            ones_c = pp.tile([128, 1], BF16, name="ones_c", tag="ones_c")
            nc.vector.tensor_copy(ones_c[:], onesf[:, 0:1])
            # all-ones [65,64]; row 64 is the partition-64-aligned stationary
            # for the denominator-broadcast matmul
            ones65 = pp.tile([65, 64], BF16, name="ones65", tag="ones65")
            nc.vector.tensor_copy(ones65[:], onesf[0:65, :])

            # ---------- PSUM slots ----------
            psS = [ps.tile([128, 1024], F32, name=f"psS{i}", tag=f"psS{i}")
                   for i in range(2)]
            av = [ps.tile([65, 512], F32, name=f"av{i}", tag=f"av{i}")
                  for i in range(2)]
            bcp = ps.tile([128, 512], F32, name="bcp", tag="bcp")
            pqp = ps.tile([128, 512], F32, name="pqp", tag="pqp")

            def ps_slot(i):
                """Six [128,512] qkv psum regions for one t-quarter."""
                if i < 4:
                    return psS[i // 2][:, (i % 2) * 512:(i % 2 + 1) * 512]
                return (bcp if i == 4 else pqp)[:]

            # ---------- activation chunks ----------
            qtc = [pp.tile([128, 512], BF16, name=f"qtc{i}", tag=f"qtc{i}")
                   for i in range(NCH)]
            ktc = [pp.tile([128, 512], BF16, name=f"ktc{i}", tag=f"ktc{i}")
                   for i in range(NCH)]
            vtc = [pp.tile([128, 512], F32, name=f"vtc{i}", tag=f"vtc{i}")
                   for i in range(NCH)]
            fdst = [qtc, ktc, vtc]
            v_sb = [pp.tile([128, 130], BF16, name=f"v{kb}", tag=f"v{kb}")
                    for kb in range(BT // 128)]

            # ---------- phase 1 pieces (also used as attention filler) ----
            def load_xt(th):
                xt = []
                for c in range(8):
                    t = xtp.tile([128, 1024], BF16, name=f"xt{th}{c}",
                                 tag=f"xt{c}")
                    nc.sync.dma_start(t[:], xT_d[c * 128:(c + 1) * 128,
                                                 th * 1024:(th + 1) * 1024])
                    xt.append(t)
                return xt

            def qkv_quantum(th, xt, f, tq, slot=None):
                # psum->sbuf copy: Scalar pre-attention (idle then), DVE
                # when running as filler inside the exp-bound windows
                eng = nc.scalar if slot is None else nc.vector
                if slot is None:
                    slot = ps_slot(f * 2 + tq)
                for c in range(8):
                    nc.tensor.matmul(
                        slot,
                        wq_sb[c][:, f * 128:(f + 1) * 128],
                        xt[c][:, tq * 512:(tq + 1) * 512],
                        start=(c == 0), stop=(c == 7),
                    )
                ch = th * 2 + tq
                if eng is nc.scalar:
                    eng.copy(fdst[f][ch][:], slot)
                else:
                    eng.tensor_copy(fdst[f][ch][:], slot)

            def rope_quantum(ch, slots=None):
                """RoPE in place on q and k chunk ch. Because the rope
                tables repeat across the rotate halves (sin[perm(d)] ==
                sin[d]), rot_half(x)*sin == perm @ (x*sin), so the sin
                multiply happens on SBUF bf16 before the matmul."""
                if slots is None:
                    slots = [pqp[:], pqp[:]]
                for which, tcl in ((0, qtc), (1, ktc)):
                    pr = slots[which]
                    xs = wk.tile([128, 512], BF16, name=f"xs{which}{ch}",
                                 tag="xs")
                    nc.vector.tensor_mul(
                        xs[:], tcl[ch][:], sin_sb[:, ch * 512:(ch + 1) * 512])
                    nc.tensor.matmul(pr, perm_sb[:], xs[:],
                                     start=True, stop=True)
                    nc.vector.tensor_mul(
                        tcl[ch][:], tcl[ch][:],
                        cos_sb[:, ch * 512:(ch + 1) * 512])
                    nc.vector.tensor_add(tcl[ch][:], tcl[ch][:], pr)

            def vt_quantum(ch, slots=None, eng=None):
                """Transpose v chunk ch into 4 v_sb key blocks. eng: engine
                for psum->sbuf copies (scalar pre-attention only — its
                copies are safe before the first exp's table load)."""
                if slots is None:
                    slots = [bcp[:, 0:128], bcp[:, 128:256]]
                if eng is None:
                    eng = nc.vector
                for j in range(4):
                    kb = ch * 4 + j
                    pv = slots[j % len(slots)]
                    nc.tensor.transpose(pv,
                                        vtc[ch][:, j * 128:(j + 1) * 128],
                                        id_sb[:])
                    v = v_sb[kb]
                    if eng is nc.scalar:
                        eng.copy(v[:, 0:64], pv[:, 0:64])
                        eng.copy(v[:, 65:129], pv[:, 64:128])
                    else:
                        eng.tensor_copy(v[:, 0:64], pv[:, 0:64])
                        eng.tensor_copy(v[:, 65:129], pv[:, 64:128])
                    nc.gpsimd.tensor_copy(v[:, 64:65], ones_c[:])
                    nc.gpsimd.tensor_copy(v[:, 129:130], ones_c[:])

            # ---------- attention ----------
            pt_tiles = [ptp.tile([128, 1024], BF16, name=f"pt{i}",
                                 tag=f"pt{i}") for i in range(6)]
            pt_idx = [0]

            def attn_head(b, h, filler):
                hp = h * 64
                pending_norm = [None]

                def s_pair(pi, qc, kb_a, kb_b):
                    """S + exp for key blocks kb_a, kb_b of query chunk qc.
                    kb_b may be None. Returns (pt, [(kb, off, F), ...])."""
                    sp = psS[pi % 2]
                    pt = pt_tiles[pt_idx[0] % 6]
                    pt_idx[0] += 1
                    ent = []
                    for slot_i, kb in enumerate((kb_a, kb_b)):
                        if kb is None:
                            continue
                        koff = max(0, (kb - 4 * qc) * 128)
                        F = 512 - koff
                        off = slot_i * 512
                        kch = ktc[b * 4 + kb // 4]
                        nc.tensor.matmul(
                            sp[:, off:off + F],
                            kch[hp:hp + 64, (kb % 4) * 128:(kb % 4 + 1) * 128],
                            qtc[b * 4 + qc][hp:hp + 64, koff:512],
                            start=True, stop=True,
                        )
                        ent.append((kb, off, F, koff))
                    diag = ent[0][3] > 0 or (len(ent) > 1 and ent[1][3] > 0) \
                        or any(kb >= 4 * qc for kb, _, _, _ in ent)
                    if not diag:
                        nc.scalar.activation(
                            pt[:, 0:1024], sp[:, 0:1024],
                            mybir.ActivationFunctionType.Exp, scale=0.125)
                    else:
                        for kb, off, F, koff in ent:
                            nc.scalar.activation(
                                pt[:, off:off + F], sp[:, off:off + F],
                                mybir.ActivationFunctionType.Exp, scale=0.125)
                            if kb >= 4 * qc:
                                nc.vector.tensor_mul(
                                    pt[:, off:off + 128], pt[:, off:off + 128],
                                    tri_sb[:])
                    if DEBUG and b == 0 and h == 0 and qc == 0 and pi == 0:
                        nc.sync.dma_start(dbg['pt0'][:], pt[:])
                    return (pt, ent)

                def av_pair(avq, qc, pair, nkb):
                    pt, ent = pair
                    for kb, off, F, koff in ent:
                        nc.tensor.matmul(
                            avq[:, koff:512],
                            v_sb[b * 16 + kb][:, h * 65:(h + 1) * 65],
                            pt[:, off:off + F],
                            start=(kb == 0), stop=(kb == nkb - 1),
                        )

                for qc in range(4):
                    nkb = 4 * qc + 4
                    avq = av[qc % 2]
                    pairs = []
                    for p in range(0, nkb, 2):
                        pairs.append((p // 2, qc, p,
                                      p + 1 if p + 1 < nkb else None))
                    # emit S pairs and AV pairs in groups of 2 pairs (4 kb),
                    # AV lagging one group behind S
                    done = []
                    groups = [pairs[g:g + 2] for g in range(0, len(pairs), 2)]
                    prev = None
                    for gi, grp in enumerate(groups):
                        cur = [s_pair(*args) for args in grp]
                        if pending_norm[0] is not None:
                            pending_norm[0]()
                            pending_norm[0] = None
                        if prev is not None:
                            for pr_ in prev:
                                av_pair(avq, qc, pr_, nkb)
                        if filler:
                            filler(qc)
                        prev = cur
                    for pr_ in prev:
                        av_pair(avq, qc, pr_, nkb)

                    def norm(b=b, h=h, qc=qc, avq=avq):
                        if DEBUG and b == 0 and h == 0 and qc == 0:
                            avf = wk.tile([65, 512], F32, name="avf",
                                          tag="avf")
                            nc.vector.tensor_copy(avf[:], avq[:])
                            nc.sync.dma_start(dbg['av0'][:], avf[:])
                        # den (psum row 64) -> sbuf -> DRAM -> broadcast
                        # read back on partitions 0:64. Keeps the PE
                        # stream free of odd-shaped broadcast matmuls;
                        # the DMA latency hides in the deferred normalize.
                        # (DVE, not Scalar: a scalar.copy here would
                        # thrash the ACT table set against exp.)
                        dsb = wk.tile([65, 512], F32, name=f"ds{b}{h}{qc}",
                                      tag="dsb")
                        nc.vector.tensor_copy(dsb[64:65, :], avq[64:65, :])
                        nc.sync.dma_start(den_dram[b, h, qc][:],
                                          dsb[64:65, :])
                        bcd = wk.tile([64, 512], F32, name=f"bcd{b}{h}{qc}",
                                      tag="bcd")
                        nc.sync.dma_start(
                            bcd[:],
                            den_dram[b, h, qc][0:1, :].broadcast_to(
                                (64, 512)))
                        bcs = wk.tile([64, 512], F32, name=f"bcs{b}{h}{qc}",
                                      tag="bcs")
                        scr = wk.tile([64, 512], F32, name=f"scr{b}{h}{qc}",
                                      tag="scr")
                        nc.vector.reciprocal_approx_accurate(
                            out=bcs[:], in_=bcd[:], scratch=scr[:])
                        if DEBUG and b == 0 and h == 0 and qc == 0:
                            nc.sync.dma_start(dbg['rc0'][:], bcs[0:1, :])
                            nc.sync.dma_start(dbg['bcs0'][:], bcs[:])
                        st = wk.tile([64, 512], BF16, name=f"st{b}{h}{qc}",
                                     tag="st")
                        nc.vector.tensor_mul(st[:], avq[0:64, :], bcs[:])
                        if DEBUG and b == 0 and h == 0 and qc == 0:
                            nc.sync.dma_start(dbg['st0'][:], st[:])
                        for half in range(2):
                            if b == 0:
                                dst, r0 = a2a_in[0], 128 * (2 * qc + half) \
                                    + 64 * h
                            else:
                                dst, r0 = a2a_in[1, h], 64 * (2 * qc + half)
                            nc.sync.dma_start(
                                dst[r0:r0 + 64, :],
                                st[:, half * 256:(half + 1) * 256])
                    pending_norm[0] = norm
                pending_norm[0]()

            def a2a_issue(key):
                nc.gpsimd.collective_compute(
                    "AllToAll",
                    mybir.AluOpType.bypass,
                    replica_groups=[list(range(NC))],
                    ins=[a2a_in[key][:]],
                    outs=[a2a_out[key][:]],
                )

            # ---------- out-projection for this core's tokens ----------
            def proj_load(b):
                pl = []
                for c in range(8):
                    t = wk.tile([128, 256], BF16, name=f"pl{b}{c}",
                                tag=f"pl{c}")
                    if b == 0:
                        nc.gpsimd.dma_start(
                            t[:], a2a_out[0][c * 128:(c + 1) * 128, :])
                    else:
                        for h in range(2):
                            nc.gpsimd.dma_start(
                                t[h * 64:(h + 1) * 64, :],
                                a2a_out[1, h][c * 64:(c + 1) * 64, :])
                    pl.append(t)
                return pl

            def proj_quantum(b, pl, tb, oh, slot=None):
                po = slot if slot is not None else pqp[:]
                for c in range(8):
                    nc.tensor.matmul(
                        po,
                        pl[c][:, tb * 128:(tb + 1) * 128],
                        wp_sb[c][:, oh * 512:(oh + 1) * 512],
                        start=(c == 0), stop=(c == 7),
                    )
                pf = wk.tile([128, 512], F32, name=f"pf{b}{tb}{oh}",
                             tag="pf")
                nc.vector.tensor_copy(pf[:], po)
                nc.sync.dma_start(
                    out_d[b * 256 + tb * 128:b * 256 + (tb + 1) * 128,
                          oh * 512:(oh + 1) * 512], pf[:])

            # ================= schedule =================
            # phase 1 for batch 0 (th 0, 1)
            xt0 = load_xt(0)
            xt1 = load_xt(1)
            for f in range(3):
                for tq in range(2):
                    qkv_quantum(0, xt0, f, tq)
            for f in range(3):
                for tq in range(2):
                    qkv_quantum(1, xt1, f, tq)
            for ch in range(4):
                rope_quantum(ch, slots=[psS[0][:, 0:512], psS[1][:, 0:512]])
                vt_quantum(ch, slots=[psS[0][:, 512:640], psS[1][:, 512:640],
                                      bcp[:, 0:128], pqp[:, 0:128]],
                           eng=nc.scalar)
            wp_load()
            if DEBUG:
                nc.sync.dma_start(dbg['q0'][:], qtc[0][:])
                nc.sync.dma_start(dbg['k0'][:], ktc[0][:])
                nc.sync.dma_start(dbg['v0'][:], vtc[0][:])
                nc.sync.dma_start(dbg['vsb0'][:], v_sb[0][:])

            # filler generator: batch-1 phase 1 then batch-0 projection
            fill_q = []
            xt2 = load_xt(2)
            for f in range(3):
                for tq in range(2):
                    fill_q.append(
                        lambda f=f, tq=tq: qkv_quantum(2, xt2, f, tq, pqp[:]))
            fill_q.append(lambda: rope_quantum(4))
            fill_q.append(lambda: vt_quantum(4))
            fill_q.append(lambda: rope_quantum(5))
            fill_q.append(lambda: vt_quantum(5))

            fill_q2 = []
            xt3 = [None]

            def load3():
                xt3[0] = load_xt(3)
            fill_q2.append(load3)
            for f in range(3):
                for tq in range(2):
                    fill_q2.append(
                        lambda f=f, tq=tq: qkv_quantum(3, xt3[0], f, tq,
                                                       pqp[:]))
            fill_q2.append(lambda: rope_quantum(6))
            fill_q2.append(lambda: vt_quantum(6))
            fill_q2.append(lambda: rope_quantum(7))
            fill_q2.append(lambda: vt_quantum(7))

            def mk_filler(queue, per_point, stride=1):
                state = {'n': 0}

                def filler(qc):
                    state['n'] += 1
                    if state['n'] % stride:
                        return
                    for _ in range(per_point * stride):
                        if queue:
                            queue.pop(0)()
                return filler

            def drain(queue):
                while queue:
                    queue.pop(0)()

            # batch 0 attention; fill with batch-1 phase-1 work
            fill_q.extend(fill_q2)
            attn_head(0, 0, mk_filler(fill_q, 2, stride=2))
            attn_head(0, 1, mk_filler(fill_q, 2, stride=2))
            a2a_issue(0)
            drain(fill_q)

            # batch 1 attention; fill with batch-0 projection and the
            # early (h0) half of batch-1's projection inputs
            pl0 = [None]
            fill_p = []

            def loadp():
                pl0[0] = proj_load(0)
            fill_p.append(loadp)
            for tb in range(2):
                for oh in range(2):
                    fill_p.append(
                        lambda tb=tb, oh=oh: proj_quantum(0, pl0[0], tb, oh))

            pl1 = [wk.tile([128, 256], BF16, name=f"pl1{c}", tag=f"pl{c}")
                   for c in range(8)]

            def load1_h0():
                for c in range(8):
                    nc.gpsimd.dma_start(
                        pl1[c][0:64, :], a2a_out[1, 0][c * 64:(c + 1) * 64, :])

            attn_head(1, 0, None)
            a2a_issue((1, 0))
            fill_p.append(load1_h0)
            attn_head(1, 1, mk_filler(fill_p, 1))
            a2a_issue((1, 1))
            drain(fill_p)

            # tail: h1 rows of proj inputs across three queues, proj on
            # the psS slots (free after attention) to avoid serialization
            engs = [nc.gpsimd, nc.sync, nc.scalar]
            for c in range(8):
                engs[c % 3].dma_start(
                    pl1[c][64:128, :], a2a_out[1, 1][c * 64:(c + 1) * 64, :])
            tail_slots = [psS[0][:, 0:512], psS[0][:, 512:1024],
                          psS[1][:, 0:512], psS[1][:, 512:1024]]
            for tb in range(2):
                for oh in range(2):
                    proj_quantum(1, pl1, tb, oh,
                                 slot=tail_slots[tb * 2 + oh])
            if DEBUG:
                nc.sync.dma_start(dbg['a2a0'][:], a2a_out[0][:])

    nc.finalize()
    return nc


def host_inputs(x, w_attn, w_proj):
    """Host-side sharding/layout prep. Returns per-core in_maps."""
    bf = ml_dtypes.bfloat16
    x2 = np.ascontiguousarray(x.reshape(BT, C).T).astype(bf)  # [C,BT]

    inv = 1.0 / (ROPE_BASE ** (np.arange(0, HD, 2, dtype=np.float32) / HD))
    tpos = np.arange(T, dtype=np.float32)
    freqs = tpos[:, None] * inv[None, :]                  # [T, 32]
    emb = np.concatenate([freqs, freqs], axis=-1)         # [T, 64]
    cosT = np.cos(emb).T.astype(np.float32)               # [64, T]
    sinT = np.sin(emb).T.astype(np.float32)
    cos_full = np.ascontiguousarray(np.tile(cosT, (2, B))).astype(bf)
    sin_full = np.ascontiguousarray(np.tile(sinT, (2, B))).astype(bf)

    m64 = np.zeros((HD, HD), dtype=np.float32)
    half = HD // 2
    for d in range(half):
        m64[d, d + half] = -1.0
        m64[d + half, d] = 1.0
    perm = np.zeros((128, 128), dtype=np.float32)
    perm[0:HD, 0:HD] = m64
    perm[HD:128, HD:128] = m64
    permT = np.ascontiguousarray(perm.T).astype(bf)

    qi = np.arange(128)[None, :]
    ki = np.arange(128)[:, None]
    tri = np.ascontiguousarray((qi >= ki).astype(np.float32)).astype(bf)

    identF = np.eye(128, dtype=np.float32)
    wpT = np.ascontiguousarray(w_proj.T).astype(bf)

    in_maps = []
    for i in range(NC):
        r0 = i * (HL * HD)
        wq = w_attn[r0:r0 + HL * HD, :]
        wk_ = w_attn[C + r0:C + r0 + HL * HD, :]
        wv = w_attn[2 * C + r0:2 * C + r0 + HL * HD, :]
        wqkvT = np.ascontiguousarray(
            np.concatenate([wq, wk_, wv], axis=0).T).astype(bf)
        in_maps.append({
            "xT": x2, "wqkvT": wqkvT, "wpT": wpT,
            "cosT": cos_full, "sinT": sin_full, "permT": permT,
            "tri": tri, "identF": identF,
        })
    return in_maps


_NC_CACHE = None


def _get_nc():
    global _NC_CACHE
    if _NC_CACHE is None:
        _NC_CACHE = build()
    return _NC_CACHE


def run(x, w_attn, w_proj, trace=False):
    nc = _get_nc()
    in_maps = host_inputs(np.asarray(x), np.asarray(w_attn),
                          np.asarray(w_proj))
    res = run_bass_kernel_spmd(nc, in_maps, list(range(NC)), trace=trace)
    # core i returns [512, 1024]: rows 0:256 = batch0 rows [256i, 256i+256),
    # rows 256:512 = batch1 rows [256i, 256i+256)
    out = np.empty((B, T, C), dtype=np.float32)
    piece = T // NC
    for i in range(NC):
        sh = res.results[i]["out"]
        out[0, i * piece:(i + 1) * piece] = sh[0:piece]
        out[1, i * piece:(i + 1) * piece] = sh[piece:2 * piece]
    return out, res


def kernel(x, w_attn, w_proj):
    out, _ = run(x, w_attn, w_proj, trace=False)
    return out


# revision 61
# speedup vs baseline: 1.0604x; 1.0213x over previous
"""Causal self-attention with RoPE on 8 TRN2 NeuronCores.

Head-parallel tensor parallelism: core i owns heads 2i, 2i+1. Each core
computes its slice of the qkv projection (bf16 operands, fp32 psum),
per-head causal attention in SBUF with diagonal-block slicing, then the
normalized attention outputs are exchanged with a per-(batch, head)
AllToAll so every core performs the full output projection for its own
512-token shard.

Scheduling notes:
- all matmuls are bf16 (full PE rate; fp32r is no faster and costs 2x
  DMA/SBUF), PSUM always fp32.
- S blocks for one query chunk are emitted in groups >= 4 per shape to
  amortize the PE tile-geometry switch (~100 ns when alternating).
- exp() runs on the Scalar engine only (the only engine with exp); it is
  the attention-phase co-bottleneck, so Scalar does nothing else and S
  psum tiles pair two key-blocks so one activation covers [128, 1024].
- the qkv/rope/v-transpose work for batch 1 and the batch-0 projection
  are emitted as filler inside the (Scalar-bound) attention windows of
  batch 0 / batch 1 to keep the PE p-state ramped.
"""

import ml_dtypes
import numpy as np

import concourse.bass as bass
import concourse.mybir as mybir
import concourse.tile as tile
from concourse import bacc
from concourse.bass_utils import run_bass_kernel_spmd

F32 = mybir.dt.float32
BF16 = mybir.dt.bfloat16

B, T, C = 2, 2048, 1024
H, HD = 16, 64
NC = 8
HL = H // NC          # heads per core = 2
BT = B * T            # 4096
FQKV = 3 * HL * HD    # 384 rows of w_attn per core
TSH = BT // NC        # 512 output rows per core
NCH = BT // 512       # 8 column chunks of the [*, BT] activations
ROPE_BASE = 10000.0


DEBUG = False


def build():
    nc = bacc.Bacc(None, target_bir_lowering=False)

    xT_d = nc.dram_tensor("xT", [C, BT], BF16, kind="ExternalInput")
    wq_d = nc.dram_tensor("wqkvT", [C, FQKV], BF16, kind="ExternalInput")
    wp_d = nc.dram_tensor("wpT", [C, C], BF16, kind="ExternalInput")
    cos_d = nc.dram_tensor("cosT", [128, BT], BF16, kind="ExternalInput")
    sin_d = nc.dram_tensor("sinT", [128, BT], BF16, kind="ExternalInput")
    perm_d = nc.dram_tensor("permT", [128, 128], BF16, kind="ExternalInput")
    tri_d = nc.dram_tensor("tri", [128, 128], BF16, kind="ExternalInput")
    id_d = nc.dram_tensor("identF", [128, 128], F32, kind="ExternalInput")
    out_d = nc.dram_tensor("out", [TSH, C], F32, kind="ExternalOutput")
    dbg = {}
    if DEBUG:
        dbg['q0'] = nc.dram_tensor("dbg_q0", [128, 512], BF16,
                                   kind="ExternalOutput")
        dbg['k0'] = nc.dram_tensor("dbg_k0", [128, 512], BF16,
                                   kind="ExternalOutput")
        dbg['v0'] = nc.dram_tensor("dbg_v0", [128, 512], F32,
                                   kind="ExternalOutput")
        dbg['vsb0'] = nc.dram_tensor("dbg_vsb0", [128, 130], BF16,
                                     kind="ExternalOutput")
        dbg['pt0'] = nc.dram_tensor("dbg_pt0", [128, 1024], BF16,
                                    kind="ExternalOutput")
        dbg['av0'] = nc.dram_tensor("dbg_av0", [65, 512], F32,
                                    kind="ExternalOutput")
        dbg['st0'] = nc.dram_tensor("dbg_st0", [64, 512], BF16,
                                    kind="ExternalOutput")
        dbg['rc0'] = nc.dram_tensor("dbg_rc0", [1, 512], F32,
                                    kind="ExternalOutput")
        dbg['bcs0'] = nc.dram_tensor("dbg_bcs0", [64, 512], F32,
                                     kind="ExternalOutput")
        dbg['a2a0'] = nc.dram_tensor("dbg_a2a0", [512, 256], BF16,
                                     kind="ExternalOutput")

    den_dram = {(b_, h_, qc_): nc.dram_tensor(f"dend{b_}{h_}{qc_}",
                                              [1, 512], F32)
                for b_ in range(2) for h_ in range(2) for qc_ in range(4)}
    # AllToAll staging. Batch 0: one merged exchange (block j rows
    # 128j:128j+128 = this core's 128 channels for core j's 256 tokens).
    # Batch 1 is split per head (block j rows 64j:64j+64) so the h0
    # exchange overlaps the h1 attention and only h1's is exposed.
    a2a_in = {0: nc.dram_tensor("a2ain0", [NC * 128, T // NC], BF16),
              (1, 0): nc.dram_tensor("a2ain10", [NC * HD, T // NC], BF16),
              (1, 1): nc.dram_tensor("a2ain11", [NC * HD, T // NC], BF16)}
    a2a_out = {0: nc.dram_tensor("a2aout0", [NC * 128, T // NC], BF16),
               (1, 0): nc.dram_tensor("a2aout10", [NC * HD, T // NC], BF16),
               (1, 1): nc.dram_tensor("a2aout11", [NC * HD, T // NC], BF16)}
    # tiny warm-up collective: its barrier absorbs the inter-core launch
    # skew while phase 1 computes, so the real exchanges don't pay it
    warm_in = nc.dram_tensor("warm_in", [NC, 8], F32)
    warm_out = nc.dram_tensor("warm_out", [NC, 8], F32)

    with tile.TileContext(nc) as tc:
        with (
            tc.tile_pool(name="persist", bufs=1) as pp,
            tc.tile_pool(name="work", bufs=2) as wk,
            tc.tile_pool(name="xtp", bufs=2) as xtp,
            tc.tile_pool(name="ptp", bufs=1) as ptp,
            tc.tile_pool(name="ps", bufs=1, space="PSUM") as ps,
        ):
            # warm-up collective first: absorbs inter-core launch skew
            nc.gpsimd.collective_compute(
                "AllToAll",
                mybir.AluOpType.bypass,
                replica_groups=[list(range(NC))],
                ins=[warm_in[:]],
                outs=[warm_out[:]],
            )

            # ---------- constants / weights ----------
            wq_sb = []
            for c in range(8):
                t = pp.tile([128, FQKV], BF16, name=f"wq{c}", tag=f"wq{c}")
                nc.gpsimd.dma_start(t[:], wq_d[c * 128:(c + 1) * 128, :])
                wq_sb.append(t)
            perm_sb = pp.tile([128, 128], BF16, name="perm_sb", tag="perm_sb")
            nc.gpsimd.dma_start(perm_sb[:], perm_d[:])
            id_sb = pp.tile([128, 128], F32, name="id_sb", tag="id_sb")
            nc.gpsimd.dma_start(id_sb[:], id_d[:])
            tri_sb = pp.tile([128, 128], BF16, name="tri_sb", tag="tri_sb")
            nc.gpsimd.dma_start(tri_sb[:], tri_d[:])
            cos_sb = pp.tile([128, BT], BF16, name="cos_sb", tag="cos_sb")
            nc.gpsimd.dma_start(cos_sb[:], cos_d[:])
            sin_sb = pp.tile([128, BT], BF16, name="sin_sb", tag="sin_sb")
            nc.gpsimd.dma_start(sin_sb[:], sin_d[:])
            # w_proj tiles declared here; DMAs deferred until phase 1 has
            # its HBM bandwidth (wp is not needed until the projection)
            wp_sb = [pp.tile([128, C], BF16, name=f"wp{c}", tag=f"wp{c}")
                     for c in range(8)]

            def wp_load():
                for c in range(8):
                    nc.gpsimd.dma_start(wp_sb[c][:],
                                        wp_d[c * 128:(c + 1) * 128, :])

            onesf = pp.tile([128, 64], F32, name="onesf", tag="onesf")
            nc.vector.memset(onesf[:], 1.0)
            ones_c = pp.tile([128, 1], BF16, name="ones_c", tag="ones_c")
            nc.vector.tensor_copy(ones_c[:], onesf[:, 0:1])
            # all-ones [65,64]; row 64 is the partition-64-aligned stationary
            # for the denominator-broadcast matmul
            ones65 = pp.tile([65, 64], BF16, name="ones65", tag="ones65")
            nc.vector.tensor_copy(ones65[:], onesf[0:65, :])

            # ---------- PSUM slots ----------
            psS = [ps.tile([128, 1024], F32, name=f"psS{i}", tag=f"psS{i}")
                   for i in range(2)]
            av = [ps.tile([65, 512], F32, name=f"av{i}", tag=f"av{i}")
                  for i in range(2)]
            bcp = ps.tile([128, 512], F32, name="bcp", tag="bcp")
            pqp = ps.tile([128, 512], F32, name="pqp", tag="pqp")

            def ps_slot(i):
                """Six [128,512] qkv psum regions for one t-quarter."""
                if i < 4:
                    return psS[i // 2][:, (i % 2) * 512:(i % 2 + 1) * 512]
                return (bcp if i == 4 else pqp)[:]

            # ---------- activation chunks ----------
            qtc = [pp.tile([128, 512], BF16, name=f"qtc{i}", tag=f"qtc{i}")
                   for i in range(NCH)]
            ktc = [pp.tile([128, 512], BF16, name=f"ktc{i}", tag=f"ktc{i}")
                   for i in range(NCH)]
            vtc = [pp.tile([128, 512], F32, name=f"vtc{i}", tag=f"vtc{i}")
                   for i in range(NCH)]
            fdst = [qtc, ktc, vtc]
            v_sb = [pp.tile([128, 130], BF16, name=f"v{kb}", tag=f"v{kb}")
                    for kb in range(BT // 128)]

            # ---------- phase 1 pieces (also used as attention filler) ----
            def load_xt(th):
                xt = []
                for c in range(8):
                    t = xtp.tile([128, 1024], BF16, name=f"xt{th}{c}",
                                 tag=f"xt{c}")
                    nc.sync.dma_start(t[:], xT_d[c * 128:(c + 1) * 128,
                                                 th * 1024:(th + 1) * 1024])
                    xt.append(t)
                return xt

            def qkv_quantum(th, xt, f, tq, slot=None):
                # psum->sbuf copy: Scalar pre-attention (idle then), DVE
                # when running as filler inside the exp-bound windows
                eng = nc.scalar if slot is None else nc.vector
                if slot is None:
                    slot = ps_slot(f * 2 + tq)
                for c in range(8):
                    nc.tensor.matmul(
                        slot,
                        wq_sb[c][:, f * 128:(f + 1) * 128],
                        xt[c][:, tq * 512:(tq + 1) * 512],
                        start=(c == 0), stop=(c == 7),
                    )
                ch = th * 2 + tq
                if eng is nc.scalar:
                    eng.copy(fdst[f][ch][:], slot)
                else:
                    eng.tensor_copy(fdst[f][ch][:], slot)

            def rope_quantum(ch, slots=None):
                """RoPE in place on q and k chunk ch. Because the rope
                tables repeat across the rotate halves (sin[perm(d)] ==
                sin[d]), rot_half(x)*sin == perm @ (x*sin), so the sin
                multiply happens on SBUF bf16 before the matmul."""
                if slots is None:
                    slots = [pqp[:], pqp[:]]
                for which, tcl in ((0, qtc), (1, ktc)):
                    pr = slots[which]
                    xs = wk.tile([128, 512], BF16, name=f"xs{which}{ch}",
                                 tag="xs")
                    nc.vector.tensor_mul(
                        xs[:], tcl[ch][:], sin_sb[:, ch * 512:(ch + 1) * 512])
                    nc.tensor.matmul(pr, perm_sb[:], xs[:],
                                     start=True, stop=True)
                    nc.vector.tensor_mul(
                        tcl[ch][:], tcl[ch][:],
                        cos_sb[:, ch * 512:(ch + 1) * 512])
                    nc.vector.tensor_add(tcl[ch][:], tcl[ch][:], pr)

            def vt_quantum(ch, slots=None, eng=None):
                """Transpose v chunk ch into 4 v_sb key blocks. eng: engine
                for psum->sbuf copies (scalar pre-attention only — its
                copies are safe before the first exp's table load)."""
                if slots is None:
                    slots = [bcp[:, 0:128], bcp[:, 128:256]]
                if eng is None:
                    eng = nc.vector
                for j in range(4):
                    kb = ch * 4 + j
                    pv = slots[j % len(slots)]
                    nc.tensor.transpose(pv,
                                        vtc[ch][:, j * 128:(j + 1) * 128],
                                        id_sb[:])
                    v = v_sb[kb]
                    if eng is nc.scalar:
                        eng.copy(v[:, 0:64], pv[:, 0:64])
                        eng.copy(v[:, 65:129], pv[:, 64:128])
                    else:
                        eng.tensor_copy(v[:, 0:64], pv[:, 0:64])
                        eng.tensor_copy(v[:, 65:129], pv[:, 64:128])
                    nc.gpsimd.tensor_copy(v[:, 64:65], ones_c[:])
                    nc.gpsimd.tensor_copy(v[:, 129:130], ones_c[:])

            # ---------- attention ----------
            pt_tiles = [ptp.tile([128, 1024], BF16, name=f"pt{i}",
                                 tag=f"pt{i}") for i in range(6)]
            pt_idx = [0]

            def attn_head(b, h, filler):
                hp = h * 64
                pending_norm = [None]

                def s_pair(pi, qc, kb_a, kb_b):
                    """S + exp for key blocks kb_a, kb_b of query chunk qc.
                    kb_b may be None. Returns (pt, [(kb, off, F), ...])."""
                    sp = psS[pi % 2]
                    pt = pt_tiles[pt_idx[0] % 6]
                    pt_idx[0] += 1
                    ent = []
                    for slot_i, kb in enumerate((kb_a, kb_b)):
                        if kb is None:
                            continue
                        koff = max(0, (kb - 4 * qc) * 128)
                        F = 512 - koff
                        off = slot_i * 512
                        kch = ktc[b * 4 + kb // 4]
                        nc.tensor.matmul(
                            sp[:, off:off + F],
                            kch[hp:hp + 64, (kb % 4) * 128:(kb % 4 + 1) * 128],
                            qtc[b * 4 + qc][hp:hp + 64, koff:512],
                            start=True, stop=True,
                        )
                        ent.append((kb, off, F, koff))
                    diag = ent[0][3] > 0 or (len(ent) > 1 and ent[1][3] > 0) \
                        or any(kb >= 4 * qc for kb, _, _, _ in ent)
                    if not diag:
                        nc.scalar.activation(
                            pt[:, 0:1024], sp[:, 0:1024],
                            mybir.ActivationFunctionType.Exp, scale=0.125)
                    else:
                        for kb, off, F, koff in ent:
                            nc.scalar.activation(
                                pt[:, off:off + F], sp[:, off:off + F],
                                mybir.ActivationFunctionType.Exp, scale=0.125)
                            if kb >= 4 * qc:
                                nc.vector.tensor_mul(
                                    pt[:, off:off + 128], pt[:, off:off + 128],
                                    tri_sb[:])
                    if DEBUG and b == 0 and h == 0 and qc == 0 and pi == 0:
                        nc.sync.dma_start(dbg['pt0'][:], pt[:])
                    return (pt, ent)

                def av_pair(avq, qc, pair, nkb):
                    pt, ent = pair
                    for kb, off, F, koff in ent:
                        nc.tensor.matmul(
                            avq[:, koff:512],
                            v_sb[b * 16 + kb][:, h * 65:(h + 1) * 65],
                            pt[:, off:off + F],
                            start=(kb == 0), stop=(kb == nkb - 1),
                        )

                for qc in range(4):
                    nkb = 4 * qc + 4
                    avq = av[qc % 2]
                    pairs = []
                    for p in range(0, nkb, 2):
                        pairs.append((p // 2, qc, p,
                                      p + 1 if p + 1 < nkb else None))
                    # emit S pairs and AV pairs in groups of 2 pairs (4 kb),
                    # AV lagging one group behind S
                    done = []
                    groups = [pairs[g:g + 2] for g in range(0, len(pairs), 2)]
                    prev = None
                    for gi, grp in enumerate(groups):
                        cur = [s_pair(*args) for args in grp]
                        if pending_norm[0] is not None:
                            pending_norm[0]()
                            pending_norm[0] = None
                        if prev is not None:
                            for pr_ in prev:
                                av_pair(avq, qc, pr_, nkb)
                        if filler:
                            filler(qc)
                        prev = cur
                    for pr_ in prev:
                        av_pair(avq, qc, pr_, nkb)

                    def norm(b=b, h=h, qc=qc, avq=avq):
                        if DEBUG and b == 0 and h == 0 and qc == 0:
                            avf = wk.tile([65, 512], F32, name="avf",
                                          tag="avf")
                            nc.vector.tensor_copy(avf[:], avq[:])
                            nc.sync.dma_start(dbg['av0'][:], avf[:])
                        # den (psum row 64) -> sbuf -> DRAM -> broadcast
                        # read back on partitions 0:64. Keeps the PE
                        # stream free of odd-shaped broadcast matmuls;
                        # the DMA latency hides in the deferred normalize.
                        # (DVE, not Scalar: a scalar.copy here would
                        # thrash the ACT table set against exp.)
                        dsb = wk.tile([65, 512], F32, name=f"ds{b}{h}{qc}",
                                      tag="dsb")
                        nc.vector.tensor_copy(dsb[64:65, :], avq[64:65, :])
                        nc.sync.dma_start(den_dram[b, h, qc][:],
                                          dsb[64:65, :])
                        bcd = wk.tile([64, 512], F32, name=f"bcd{b}{h}{qc}",
                                      tag="bcd")
                        nc.sync.dma_start(
                            bcd[:],
                            den_dram[b, h, qc][0:1, :].broadcast_to(
                                (64, 512)))
                        bcs = wk.tile([64, 512], F32, name=f"bcs{b}{h}{qc}",
                                      tag="bcs")
                        scr = wk.tile([64, 512], F32, name=f"scr{b}{h}{qc}",
                                      tag="scr")
                        nc.vector.reciprocal_approx_accurate(
                            out=bcs[:], in_=bcd[:], scratch=scr[:])
                        if DEBUG and b == 0 and h == 0 and qc == 0:
                            nc.sync.dma_start(dbg['rc0'][:], bcs[0:1, :])
                            nc.sync.dma_start(dbg['bcs0'][:], bcs[:])
                        st = wk.tile([64, 512], BF16, name=f"st{b}{h}{qc}",
                                     tag="st")
                        nc.vector.tensor_mul(st[:], avq[0:64, :], bcs[:])
                        if DEBUG and b == 0 and h == 0 and qc == 0:
                            nc.sync.dma_start(dbg['st0'][:], st[:])
                        for half in range(2):
                            if b == 0:
                                dst, r0 = a2a_in[0], 128 * (2 * qc + half) \
                                    + 64 * h
                            else:
                                dst, r0 = a2a_in[1, h], 64 * (2 * qc + half)
                            nc.sync.dma_start(
                                dst[r0:r0 + 64, :],
                                st[:, half * 256:(half + 1) * 256])
                    pending_norm[0] = norm
                pending_norm[0]()

            def a2a_issue(key):
                nc.gpsimd.collective_compute(
                    "AllToAll",
                    mybir.AluOpType.bypass,
                    replica_groups=[list(range(NC))],
                    ins=[a2a_in[key][:]],
                    outs=[a2a_out[key][:]],
                )

            # ---------- out-projection for this core's tokens ----------
            def proj_load(b):
                pl = []
                for c in range(8):
                    t = wk.tile([128, 256], BF16, name=f"pl{b}{c}",
                                tag=f"pl{c}")
                    if b == 0:
                        nc.gpsimd.dma_start(
                            t[:], a2a_out[0][c * 128:(c + 1) * 128, :])
                    else:
                        for h in range(2):
                            nc.gpsimd.dma_start(
                                t[h * 64:(h + 1) * 64, :],
                                a2a_out[1, h][c * 64:(c + 1) * 64, :])
                    pl.append(t)
                return pl

            def proj_quantum(b, pl, tb, oh, slot=None):
                po = slot if slot is not None else pqp[:]
                for c in range(8):
                    nc.tensor.matmul(
                        po,
                        pl[c][:, tb * 128:(tb + 1) * 128],
                        wp_sb[c][:, oh * 512:(oh + 1) * 512],
                        start=(c == 0), stop=(c == 7),
                    )
                pf = wk.tile([128, 512], F32, name=f"pf{b}{tb}{oh}",
                             tag="pf")
                nc.vector.tensor_copy(pf[:], po)
                nc.sync.dma_start(
                    out_d[b * 256 + tb * 128:b * 256 + (tb + 1) * 128,
                          oh * 512:(oh + 1) * 512], pf[:])

            # ================= schedule =================
            # phase 1 for batch 0 (th 0, 1)
            xt0 = load_xt(0)
            xt1 = load_xt(1)
            for f in range(3):
                for tq in range(2):
                    qkv_quantum(0, xt0, f, tq)
            for f in range(3):
                for tq in range(2):
                    qkv_quantum(1, xt1, f, tq)
            for ch in range(4):
                rope_quantum(ch, slots=[psS[0][:, 0:512], psS[1][:, 0:512]])
                vt_quantum(ch, slots=[psS[0][:, 512:640], psS[1][:, 512:640],
                                      bcp[:, 0:128], pqp[:, 0:128]],
                           eng=nc.scalar)
            wp_load()
            if DEBUG:
                nc.sync.dma_start(dbg['q0'][:], qtc[0][:])
                nc.sync.dma_start(dbg['k0'][:], ktc[0][:])
                nc.sync.dma_start(dbg['v0'][:], vtc[0][:])
                nc.sync.dma_start(dbg['vsb0'][:], v_sb[0][:])

            # filler generator: batch-1 phase 1 then batch-0 projection
            fill_q = []
            xt2 = load_xt(2)
            for f in range(3):
                for tq in range(2):
                    fill_q.append(
                        lambda f=f, tq=tq: qkv_quantum(2, xt2, f, tq, pqp[:]))
            fill_q.append(lambda: rope_quantum(4))
            fill_q.append(lambda: vt_quantum(4))
            fill_q.append(lambda: rope_quantum(5))
            fill_q.append(lambda: vt_quantum(5))

            fill_q2 = []
            xt3 = [None]

            def load3():
                xt3[0] = load_xt(3)
            fill_q2.append(load3)
            for f in range(3):
                for tq in range(2):
                    fill_q2.append(
                        lambda f=f, tq=tq: qkv_quantum(3, xt3[0], f, tq,
                                                       pqp[:]))
            fill_q2.append(lambda: rope_quantum(6))
            fill_q2.append(lambda: vt_quantum(6))
            fill_q2.append(lambda: rope_quantum(7))
            fill_q2.append(lambda: vt_quantum(7))

            def mk_filler(queue, per_point, stride=1):
                state = {'n': 0}

                def filler(qc):
                    state['n'] += 1
                    if state['n'] % stride:
                        return
                    for _ in range(per_point * stride):
                        if queue:
                            queue.pop(0)()
                return filler

            def drain(queue):
                while queue:
                    queue.pop(0)()

            # batch 0 attention; fill with batch-1 phase-1 work
            fill_q.extend(fill_q2)
            attn_head(0, 0, mk_filler(fill_q, 2, stride=2))
            attn_head(0, 1, mk_filler(fill_q, 2, stride=2))
            a2a_issue(0)
            drain(fill_q)

            # batch 1 attention; fill with batch-0 projection and the
            # early (h0) half of batch-1's projection inputs
            pl0 = [None]
            fill_p = []

            def loadp():
                pl0[0] = proj_load(0)
            fill_p.append(loadp)
            for tb in range(2):
                for oh in range(2):
                    fill_p.append(
                        lambda tb=tb, oh=oh: proj_quantum(0, pl0[0], tb, oh))

            pl1 = [wk.tile([128, 256], BF16, name=f"pl1{c}", tag=f"pl{c}")
                   for c in range(8)]

            def load1_h0():
                for c in range(8):
                    nc.gpsimd.dma_start(
                        pl1[c][0:64, :], a2a_out[1, 0][c * 64:(c + 1) * 64, :])

            attn_head(1, 0, None)
            a2a_issue((1, 0))
            fill_p.append(load1_h0)
            attn_head(1, 1, mk_filler(fill_p, 1, stride=2))
            a2a_issue((1, 1))
            drain(fill_p)

            # tail: h1 rows of proj inputs across three queues, proj on
            # the psS slots (free after attention) to avoid serialization
            engs = [nc.gpsimd, nc.sync, nc.scalar]
            for c in range(8):
                engs[c % 3].dma_start(
                    pl1[c][64:128, :], a2a_out[1, 1][c * 64:(c + 1) * 64, :])
            tail_slots = [psS[0][:, 0:512], psS[0][:, 512:1024],
                          psS[1][:, 0:512], psS[1][:, 512:1024]]
            for tb in range(2):
                for oh in range(2):
                    proj_quantum(1, pl1, tb, oh,
                                 slot=tail_slots[tb * 2 + oh])
            if DEBUG:
                nc.sync.dma_start(dbg['a2a0'][:], a2a_out[0][:])

    nc.finalize()
    return nc


def host_inputs(x, w_attn, w_proj):
    """Host-side sharding/layout prep. Returns per-core in_maps."""
    bf = ml_dtypes.bfloat16
    x2 = np.ascontiguousarray(x.reshape(BT, C).T).astype(bf)  # [C,BT]

    inv = 1.0 / (ROPE_BASE ** (np.arange(0, HD, 2, dtype=np.float32) / HD))
    tpos = np.arange(T, dtype=np.float32)
    freqs = tpos[:, None] * inv[None, :]                  # [T, 32]
    emb = np.concatenate([freqs, freqs], axis=-1)         # [T, 64]
    cosT = np.cos(emb).T.astype(np.float32)               # [64, T]
    sinT = np.sin(emb).T.astype(np.float32)
    cos_full = np.ascontiguousarray(np.tile(cosT, (2, B))).astype(bf)
    sin_full = np.ascontiguousarray(np.tile(sinT, (2, B))).astype(bf)

    m64 = np.zeros((HD, HD), dtype=np.float32)
    half = HD // 2
    for d in range(half):
        m64[d, d + half] = -1.0
        m64[d + half, d] = 1.0
    perm = np.zeros((128, 128), dtype=np.float32)
    perm[0:HD, 0:HD] = m64
    perm[HD:128, HD:128] = m64
    permT = np.ascontiguousarray(perm.T).astype(bf)

    qi = np.arange(128)[None, :]
    ki = np.arange(128)[:, None]
    tri = np.ascontiguousarray((qi >= ki).astype(np.float32)).astype(bf)

    identF = np.eye(128, dtype=np.float32)
    wpT = np.ascontiguousarray(w_proj.T).astype(bf)

    in_maps = []
    for i in range(NC):
        r0 = i * (HL * HD)
        wq = w_attn[r0:r0 + HL * HD, :]
        wk_ = w_attn[C + r0:C + r0 + HL * HD, :]
        wv = w_attn[2 * C + r0:2 * C + r0 + HL * HD, :]
        wqkvT = np.ascontiguousarray(
            np.concatenate([wq, wk_, wv], axis=0).T).astype(bf)
        in_maps.append({
            "xT": x2, "wqkvT": wqkvT, "wpT": wpT,
            "cosT": cos_full, "sinT": sin_full, "permT": permT,
            "tri": tri, "identF": identF,
        })
    return in_maps


_NC_CACHE = None


def _get_nc():
    global _NC_CACHE
    if _NC_CACHE is None:
        _NC_CACHE = build()
    return _NC_CACHE


def run(x, w_attn, w_proj, trace=False):
    nc = _get_nc()
    in_maps = host_inputs(np.asarray(x), np.asarray(w_attn),
                          np.asarray(w_proj))
    res = run_bass_kernel_spmd(nc, in_maps, list(range(NC)), trace=trace)
    # core i returns [512, 1024]: rows 0:256 = batch0 rows [256i, 256i+256),
    # rows 256:512 = batch1 rows [256i, 256i+256)
    out = np.empty((B, T, C), dtype=np.float32)
    piece = T // NC
    for i in range(NC):
        sh = res.results[i]["out"]
        out[0, i * piece:(i + 1) * piece] = sh[0:piece]
        out[1, i * piece:(i + 1) * piece] = sh[piece:2 * piece]
    return out, res


def kernel(x, w_attn, w_proj):
    out, _ = run(x, w_attn, w_proj, trace=False)
    return out
